# revision 1
# baseline (speedup 1.0000x reference)
"""ConvCNP1d Trainium2 kernel, v2: banded RBF via host-side sorting.

ls = ln2 over a 128-unit data range means exp(-0.5 d^2/ls^2) < e^-7.5
beyond |d| ~ 2.7 units, so both kernel matrices are banded once xc / xt
are sorted (host-side; the output is un-sorted at the end).

Encoder (per batch): the t-grid is cut into 8 value-blocks of 256 points.
For block k the xc's inside [t_lo - m, t_hi + m] are gathered (host) into
NCH_E[k] chunks of 128 sorted points (NCH_E = max over batches, so the
single SPMD program fits every core).  Per chunk one scalar_tensor_tensor
(DVE or Pool) writes a*t'^2 - 2a*xc'*t' into a PSUM tile [128, NCH*256];
one rank-2 PE matmul against a block-diagonal 0/1 table adds the
per-(chunk,partition) a*xc'^2 term (hi/lo fp16 split keeps it exact);
a single fused Exp activation emits the whole K tile in fp16 (one Scalar
op per block instead of per chunk); NCH accumulating matmuls against the
phi weights give the block's h [2, 256] in PSUM.

Decoder mirrors this with targets and grid swapped, but blocks are
xt *value*-blocks aligned to the grid so the window chunk indices are
compile-time constants shared by all cores: block k holds the sorted xt
falling in grid span [256k, 256(k+1)), padded to a uniform TGTU columns;
its window is the fixed grid chunks J0S[k] .. J0S[k]+NCH_D[k].  Each
block runs as two half-tiles of <=2 chunks so PSUM tiles stay <=2 banks.
Decoder [2, TGTU] outputs are DMA'd straight from PSUM to DRAM.

h0/h1 epilogue: reciprocal_approx_fast (single custom DVE op, ~18 bits).
Conv stack: 5 taps folded into the partition dim (shifted copies via
4x-mode DVE tensor_copy), so each layer is 4-8 matmuls, not 20-40.
conv4's identity/softplus epilogue runs in transposed [128, 16] layout
(PE transposes) which is also exactly the decoder's fT weight layout.
All activation functions used (Exp, Ln, Relu, Abs, Identity, Copy) are
grouped to minimize act-table reloads.
"""

import numpy as np

T_GRID = 2048
B = 16
N = 2048
NCORES = 8
BLOC = B // NCORES
NBLK = 8
WBLK = T_GRID // NBLK  # 256
ETH = 7.5              # exponent cutoff; kernel entries below e^-ETH dropped

_PROG_CACHE = {}


def build_program(cfg):
    import concourse.bacc as bacc
    import concourse.tile as tile
    from concourse import mybir

    f32 = mybir.dt.float32
    f16 = mybir.dt.float16
    AF = mybir.ActivationFunctionType
    OP = mybir.AluOpType
    import concourse.bass as bass_mod

    NCH_E = cfg["NCH_E"]
    NCH_D = cfg["NCH_D"]
    J0S = cfg["J0S"]
    TGTU = cfg["TGTU"]
    SE = sum(NCH_E)
    SD = sum(NCH_D)
    MAXNE = max(NCH_E)
    MAXND = max(NCH_D)
    os_rho = cfg["os_rho"]
    b4_0 = cfg["b4_0"]
    b4_1 = cfg["b4_1"]
    FRAC = cfg["stt_dve_frac"]
    assert MAXNE * WBLK <= 1024 and 2 * TGTU <= 1024

    nc = bacc.Bacc(None, target_bir_lowering=False)

    TE0h = nc.declare_dram_parameter("TE0", [1, WBLK], f32, isOutput=False)
    BDEh = nc.declare_dram_parameter("BDE", [2 * MAXNE + 2, MAXNE * WBLK], f16, isOutput=False)
    TCh = nc.declare_dram_parameter("TCONV", [16, T_GRID], f32, isOutput=False)
    W1h = nc.declare_dram_parameter("W1f", [2, 80], f16, isOutput=False)
    W2h = nc.declare_dram_parameter("W2f", [16, 160], f16, isOutput=False)
    W3h = nc.declare_dram_parameter("W3f", [128, 16], f16, isOutput=False)
    W3eh = nc.declare_dram_parameter("W3e", [32, 16], f16, isOutput=False)
    W4h = nc.declare_dram_parameter("W4f", [16, 10], f16, isOutput=False)
    B2h = nc.declare_dram_parameter("B2", [32, 1], f32, isOutput=False)
    B3h = nc.declare_dram_parameter("B3", [16, 1], f32, isOutput=False)
    ID2h = nc.declare_dram_parameter("ID2", [2, 2], f16, isOutput=False)
    XCSh = nc.declare_dram_parameter("XCS", [BLOC, 128, SE], f32, isOutput=False)
    PHIh = nc.declare_dram_parameter("PHI", [BLOC, 128, 2 * SE], f16, isOutput=False)
    XCBh = nc.declare_dram_parameter("XCB", [BLOC, 2 * MAXNE + 2, NBLK * 128], f16, isOutput=False)
    XTPh = nc.declare_dram_parameter("XTP", [BLOC, NBLK, TGTU], f32, isOutput=False)
    TDSh = nc.declare_dram_parameter("TDS", [BLOC, 128, SD], f32, isOutput=False)
    TDB0h = nc.declare_dram_parameter("TDB0", [BLOC, 6, NBLK * 128], f16, isOutput=False)
    TDB1h = nc.declare_dram_parameter("TDB1", [BLOC, 6, NBLK * 128], f16, isOutput=False)
    XTQ2h = nc.declare_dram_parameter("XTQ2", [BLOC, 6, NBLK * 2 * TGTU], f16, isOutput=False)
    OUTh = nc.declare_dram_parameter("out", [BLOC, 2, NBLK * TGTU], f32, isOutput=True)

    def bcast(dst, src_ap, n):
        nc.sync.dma_start(out=dst, in_=bass_mod.AP(
            tensor=src_ap.tensor, offset=src_ap.offset,
            ap=[[0, 128], [1, n]]))

    with tile.TileContext(nc) as tc:
        with (
            tc.tile_pool(name="singles", bufs=1) as singles,
            tc.tile_pool(name="perb", bufs=2) as perb,
            tc.tile_pool(name="kpool", bufs=3) as kpool,
            tc.tile_pool(name="k2keep", bufs=1) as k2keep,
            tc.tile_pool(name="small", bufs=1) as small,
            tc.tile_pool(name="psE", bufs=2, space="PSUM") as psE,
            tc.tile_pool(name="psC", bufs=2, space="PSUM") as psC,
            tc.tile_pool(name="psH", bufs=2, space="PSUM") as psH,
        ):
            TE0_sb = singles.tile([128, WBLK], f32)
            bcast(TE0_sb, TE0h[:, :], WBLK)
            BDE_sb = singles.tile([2 * MAXNE + 2, MAXNE * WBLK], f16)
            nc.sync.dma_start(out=BDE_sb, in_=BDEh[:, :])
            TC_sb = singles.tile([16, T_GRID], f32)
            nc.sync.dma_start(out=TC_sb, in_=TCh[:, :])
            W1_sb = singles.tile([2, 80], f16)
            nc.sync.dma_start(out=W1_sb, in_=W1h[:, :])
            W2_sb = singles.tile([16, 160], f16)
            nc.sync.dma_start(out=W2_sb, in_=W2h[:, :])
            W3_sb = singles.tile([128, 16], f16)
            nc.sync.dma_start(out=W3_sb, in_=W3h[:, :])
            W3e_sb = singles.tile([32, 16], f16)
            nc.sync.dma_start(out=W3e_sb, in_=W3eh[:, :])
            W4_sb = singles.tile([16, 10], f16)
            nc.sync.dma_start(out=W4_sb, in_=W4h[:, :])
            B2_sb = singles.tile([32, 1], f32)
            nc.sync.dma_start(out=B2_sb, in_=B2h[:, :])
            B3_sb = singles.tile([16, 1], f32)
            nc.sync.dma_start(out=B3_sb, in_=B3h[:, :])
            ID2_sb = singles.tile([2, 2], f16)
            nc.sync.dma_start(out=ID2_sb, in_=ID2h[:, :])

            st = [dict() for _ in range(BLOC)]

            def loads(b):
                s = st[b]
                s["XCS"] = perb.tile([128, SE], f32, tag="XCS", name="XCS_sb")
                nc.sync.dma_start(out=s["XCS"], in_=XCSh[b])
                s["PHI"] = perb.tile([128, 2 * SE], f16, tag="PHI", name="PHI_sb")
                nc.sync.dma_start(out=s["PHI"], in_=PHIh[b])
                s["XCB"] = perb.tile([2 * MAXNE + 2, NBLK * 128], f16, tag="XCB", name="XCB_sb")
                nc.sync.dma_start(out=s["XCB"], in_=XCBh[b])
                s["TDS"] = perb.tile([128, SD], f32, tag="TDS", name="TDS_sb")
                nc.sync.dma_start(out=s["TDS"], in_=TDSh[b])
                s["TDB0"] = perb.tile([6, NBLK * 128], f16, tag="TDB0", name="TDB0_sb")
                nc.sync.dma_start(out=s["TDB0"], in_=TDB0h[b])
                s["TDB1"] = perb.tile([6, NBLK * 128], f16, tag="TDB1", name="TDB1_sb")
                nc.sync.dma_start(out=s["TDB1"], in_=TDB1h[b])
                s["XTQ2"] = perb.tile([6, NBLK * 2 * TGTU], f16, tag="XTQ2", name="XTQ2_sb")
                nc.sync.dma_start(out=s["XTQ2"], in_=XTQ2h[b])
                for k in range(NBLK):
                    xp = perb.tile([128, TGTU], f32, tag=f"xtp{k}", name=f"xtp{k}")
                    bcast(xp, XTPh[b, k], TGTU)
                    s[f"xtp{k}"] = xp
                s["h"] = perb.tile([2, T_GRID], f32, tag="h_sb", name="h_sb")
                s["rep2"] = perb.tile([2, T_GRID + 4], f16, tag="rep2", name="rep2")
                nc.vector.memset(s["rep2"][:, 0:2], 0.0)
                nc.vector.memset(s["rep2"][:, T_GRID + 2:T_GRID + 4], 0.0)
                s["f1x5"] = perb.tile([16, T_GRID + 4], f16, tag="f1x5", name="f1x5")
                nc.vector.memset(s["f1x5"][0:16, 0:2], 0.0)
                nc.vector.memset(s["f1x5"][0:16, T_GRID + 2:T_GRID + 4], 0.0)
                s["f2x4"] = perb.tile([128, T_GRID + 4], f16, tag="f2x4", name="f2x4")
                nc.vector.memset(s["f2x4"][0:32, 0:2], 0.0)
                nc.vector.memset(s["f2x4"][0:32, T_GRID + 2:T_GRID + 4], 0.0)
                s["f3x5"] = perb.tile([16, T_GRID + 4], f16, tag="f3x5", name="f3x5")
                nc.vector.memset(s["f3x5"][0:16, 0:2], 0.0)
                nc.vector.memset(s["f3x5"][0:16, T_GRID + 2:T_GRID + 4], 0.0)
                s["fraw"] = perb.tile([2, T_GRID], f16, tag="fraw", name="fraw")
                s["fT"] = perb.tile([128, 2, 16], f16, tag="fT", name="fT")

            def kgen(in0, scal, base, nch, w, wb, bd, ktile, kslice0):
                """Rank-(2nch+2) PE matmul writes the per-(chunk,partition)
                bias plus the squared-coordinate row into PSUM (start=True);
                then per-chunk DVE STTs do E += in0*scal (RMW of the PE-
                written PSUM, the same pattern the conv TCONV add uses);
                one fused Exp emits the fp16 K tile."""
                tot = nch * w
                rows = 2 * nch + 2
                eps = psE.tile([128, 1024], f32, tag="E", name="E_ps")
                splits = ([(0, tot)] if tot <= 512
                          else [(0, 512), (512, tot)])
                for (c0, c1) in splits:
                    nc.tensor.matmul(
                        eps[:, c0:c1],
                        wb[0:rows, :],
                        bd[0:rows, c0:c1],
                        start=True, stop=True,
                    )
                for c in range(nch):
                    sl = slice(w * c, w * (c + 1))
                    nc.vector.scalar_tensor_tensor(
                        eps[:, sl],
                        in0, scal[:, base + c:base + c + 1], eps[:, sl],
                        OP.mult, OP.add,
                    )
                nc.scalar.activation(
                    out=ktile[:, kslice0:kslice0 + tot],
                    in_=eps[:, 0:tot], func=AF.Exp)

            def enc_block(b, k):
                s = st[b]
                nch = NCH_E[k]
                base = sum(NCH_E[:k])
                kt = kpool.tile([128, MAXNE * WBLK], f16, tag="K", name="K1t")
                kgen(TE0_sb, s["XCS"], base, nch, WBLK,
                     s["XCB"][:, 128 * k:128 * (k + 1)], BDE_sb, kt, 0)
                hps = psH.tile([2, WBLK], f32, tag="hms", name="h_ps")
                for c in range(nch):
                    nc.tensor.matmul(
                        hps,
                        s["PHI"][:, 2 * (base + c):2 * (base + c) + 2],
                        kt[:, WBLK * c:WBLK * (c + 1)],
                        start=(c == 0), stop=(c == nch - 1),
                    )
                nc.vector.tensor_copy(s["h"][:, WBLK * k:WBLK * (k + 1)], hps)

            def dec_half(b, k, half):
                s = st[b]
                nch = min(2, NCH_D[k] - 2 * half)
                base = sum(NCH_D[:k]) + 2 * half
                if half == 0:
                    s[f"k2t_{k}"] = k2keep.tile(
                        [128, MAXND * TGTU], f16, tag=f"k2_{b}_{k}",
                        name=f"k2_{b}_{k}")
                tdb = s["TDB0"] if half == 0 else s["TDB1"]
                kgen(s[f"xtp{k}"], s["TDS"], base, nch, TGTU,
                     tdb[:, 128 * k:128 * (k + 1)],
                     s["XTQ2"][:, 2 * TGTU * k:2 * TGTU * (k + 1)],
                     s[f"k2t_{k}"], 2 * half * TGTU)

            def epilogue(b):
                # row-1 reads/writes need DMA (compute engines are limited
                # to partition bases 0/32/64/96); latency is covered by the
                # other batch's encoder work in the emission order.
                s = st[b]
                rec = small.tile([1, T_GRID], f32, tag="rec", name="rec")
                h1 = small.tile([1, T_GRID], f32, tag="h1", name="h1")
                ratf = small.tile([1, T_GRID], f16, tag="ratf", name="ratf")
                nc.sync.dma_start(out=h1, in_=s["h"][1:2, :])
                nc.vector.reciprocal_approx_fast(rec, s["h"][0:1, :])
                nc.vector.tensor_mul(ratf, h1, rec)
                nc.sync.dma_start(out=s["rep2"][1:2, 2:2 + T_GRID], in_=ratf)
                nc.scalar.copy(s["rep2"][0:1, 2:2 + T_GRID], s["h"][0:1, :])

            def conv_layer(b, l):
                # taps fold into partitions only where the shifted copies
                # land on legal 32-aligned partition bases (conv3: 32ch);
                # conv1/2/4 run 5 accumulating tap-matmuls per chunk.
                s = st[b]
                if l == 0:
                    w_sb, it, O, taps = W1_sb, s["rep2"], 16, 5
                elif l == 1:
                    w_sb, it, O, taps = W2_sb, s["f1x5"], 32, 5
                elif l == 2:
                    for o in range(1, 4):
                        nc.vector.tensor_copy(
                            s["f2x4"][32 * o:32 * o + 32, 0:T_GRID + 4 - o],
                            s["f2x4"][0:32, o:T_GRID + 4])
                    w_sb, it, O, taps = W3_sb, s["f2x4"], 16, 0
                else:
                    w_sb, it, O, taps = W4_sb, s["f3x5"], 2, 5
                for n in range(4):
                    c0 = 512 * n
                    sl = slice(c0, c0 + 512)
                    ps = psC.tile([O, 512], f32, tag="c", name="c_ps")
                    if l == 2:
                        nc.tensor.matmul(ps, w_sb, it[:, sl],
                                         start=True, stop=False)
                        nc.tensor.matmul(ps, W3e_sb, it[0:32, c0 + 4:c0 + 516],
                                         start=False, stop=True)
                    else:
                        ni, no = {0: (2, 16), 1: (16, 32), 3: (16, 2)}[l]
                        for o in range(5):
                            nc.tensor.matmul(
                                ps, w_sb[:, no * o:no * (o + 1)],
                                it[0:ni, c0 + o:c0 + o + 512],
                                start=(o == 0), stop=(o == 4))
                    if l == 0:
                        nc.vector.tensor_add(ps, ps, TC_sb[:, sl])
                        nc.scalar.activation(
                            out=s["f1x5"][0:16, 2 + c0:2 + c0 + 512],
                            in_=ps, func=AF.Relu)
                    elif l == 1:
                        nc.scalar.activation(
                            out=s["f2x4"][0:32, 2 + c0:2 + c0 + 512],
                            in_=ps, func=AF.Relu, bias=B2_sb)
                    elif l == 2:
                        nc.scalar.activation(
                            out=s["f3x5"][0:16, 2 + c0:2 + c0 + 512],
                            in_=ps, func=AF.Relu, bias=B3_sb)
                    else:
                        nc.vector.tensor_copy(s["fraw"][:, sl], ps)

            def fchain(b):
                s = st[b]
                ftp = psC.tile([128, 32], f16, tag="c", name="ftp")
                for j in range(16):
                    nc.tensor.transpose(
                        ftp[:, 2 * j:2 * j + 2],
                        s["fraw"][:, 128 * j:128 * (j + 1)],
                        ID2_sb)
                t1 = small.tile([128, 16], f32, tag="t1", name="t1")
                t4 = small.tile([128, 16], f32, tag="t4", name="t4")
                nc.scalar.activation(
                    out=s["fT"][:, 0, :], in_=ftp[:, 0::2], func=AF.Identity,
                    scale=float(os_rho), bias=float(os_rho * b4_0))
                nc.scalar.activation(out=t1, in_=ftp[:, 1::2], func=AF.Abs,
                                     bias=float(b4_1))
                nc.scalar.activation(out=t1, in_=t1, func=AF.Exp, scale=-1.0)
                nc.scalar.activation(out=t1, in_=t1, func=AF.Ln, bias=1.0)
                nc.scalar.activation(out=t4, in_=ftp[:, 1::2], func=AF.Relu,
                                     scale=float(os_rho),
                                     bias=float(os_rho * b4_1))
                nc.vector.scalar_tensor_tensor(
                    s["fT"][:, 1, :], t1, float(os_rho), t4, OP.mult, OP.add)

            def dec_mm(b, k):
                s = st[b]
                kt = s[f"k2t_{k}"]
                nch = NCH_D[k]
                msps = psH.tile([2, TGTU], f32, tag="hms", name="ms_ps")
                for c in range(nch):
                    nc.tensor.matmul(
                        msps,
                        s["fT"][:, :, J0S[k] + c],
                        kt[:, TGTU * c:TGTU * (c + 1)],
                        start=(c == 0), stop=(c == nch - 1),
                    )
                osl = small.tile([2, TGTU], f32, tag="osb", name="osb",
                                 bufs=3)
                nc.vector.tensor_copy(osl, msps)
                nc.sync.dma_start(
                    out=OUTh[b, :, TGTU * k:TGTU * (k + 1)], in_=osl)

            # ---------------- emission ----------------
            loads(0)
            loads(1)
            for k in range(NBLK):
                enc_block(0, k)
            for k in range(NBLK):
                enc_block(1, k)
            epilogue(0)

            dec_units = [(b, k, h) for b in range(BLOC)
                         for k in range(NBLK)
                         for h in range(2) if 2 * h < NCH_D[k]]
            conv_units = [(0, 0), (1, 0), (0, 1), (1, 1),
                          (0, 2), (1, 2), (0, 3), (1, 3)]
            per = (len(dec_units) + len(conv_units) - 1) // len(conv_units)
            du = 0
            for i, (cb, cl) in enumerate(conv_units):
                if cb == 1 and cl == 0:
                    epilogue(1)
                conv_layer(cb, cl)
                for _ in range(per):
                    if du < len(dec_units):
                        b, k, h = dec_units[du]
                        dec_half(b, k, h)
                        du += 1
            while du < len(dec_units):
                b, k, h = dec_units[du]
                dec_half(b, k, h)
                du += 1

            fchain(0)
            for k in range(NBLK):
                dec_mm(0, k)
            fchain(1)
            for k in range(NBLK):
                dec_mm(1, k)

    nc.compile()
    return nc


def make_inmaps(inputs):
    f32 = np.float32
    f16 = np.float16
    f64 = np.float64
    xc = np.asarray(inputs["xc"])[..., 0].astype(f32)
    yc = np.asarray(inputs["yc"])[..., 0].astype(f32)
    xt = np.asarray(inputs["xt"])[..., 0].astype(f32)
    ls_psi = f64(np.float32(inputs["ls_psi"]))
    os_psi = f64(np.float32(inputs["os_psi"]))
    ls_rho = f64(np.float32(inputs["ls_rho"]))
    os_rho = f64(np.float32(inputs["os_rho"]))
    w = [np.asarray(inputs[f"w{i}"]).astype(f32) for i in (1, 2, 3, 4)]
    bs = [np.asarray(inputs[f"b{i}"]).astype(f32) for i in (1, 2, 3, 4)]

    lower = np.minimum(xc.min(), xt.min())
    upper = np.maximum(xc.max(), xt.max())
    t64 = np.linspace(f64(lower), f64(upper), T_GRID)
    delta = (t64[-1] - t64[0]) / (T_GRID - 1)

    a_psi = -0.5 / (ls_psi * ls_psi)
    a_rho = -0.5 / (ls_rho * ls_rho)
    m_psi = np.sqrt(ETH / -a_psi)
    m_rho = np.sqrt(ETH / -a_rho)
    MPTS = int(np.ceil(m_rho / delta))

    perm_c = np.argsort(xc, axis=1, kind="stable")
    xcs = np.take_along_axis(xc, perm_c, 1).astype(f64)
    ycs = np.take_along_axis(yc, perm_c, 1).astype(f64)
    perm_t = np.argsort(xt, axis=1, kind="stable")
    xts = np.take_along_axis(xt, perm_t, 1).astype(f64)

    # encoder windows
    eidx = np.zeros((B, NBLK, 2), np.int64)
    for k in range(NBLK):
        lo = t64[WBLK * k] - m_psi
        hi = t64[WBLK * (k + 1) - 1] + m_psi
        for b in range(B):
            eidx[b, k, 0] = np.searchsorted(xcs[b], lo)
            eidx[b, k, 1] = np.searchsorted(xcs[b], hi)
    ecnt = eidx[:, :, 1] - eidx[:, :, 0]
    NCH_E = [max(1, int(np.ceil(ecnt[:, k].max() / 128))) for k in range(NBLK)]
    assert max(NCH_E) <= 4, NCH_E

    # decoder quantile-blocks: 256 sorted targets each; window chunks are
    # derived from the extreme quantiles over ALL batches so the single
    # program covers every core.
    TGTU = WBLK
    J0S, J1S = [], []
    for k in range(NBLK):
        xmin = min(xts[b, WBLK * k] for b in range(B))
        xmax = max(xts[b, WBLK * (k + 1) - 1] for b in range(B))
        g0 = max(0, int(np.searchsorted(t64, xmin - m_rho)) - 1)
        g1 = min(T_GRID - 1, int(np.searchsorted(t64, xmax + m_rho)))
        j0 = g0 // 128
        j1 = g1 // 128 + 1
        J0S.append(j0)
        J1S.append(j1)
    NCH_D = [J1S[k] - J0S[k] for k in range(NBLK)]
    assert max(NCH_D) <= 4, NCH_D
    tsplit = [np.arange(NBLK + 1) * WBLK for _ in range(B)]
    SE = sum(NCH_E)
    SD = sum(NCH_D)
    MAXNE = max(NCH_E)
    MAXND = max(NCH_D)

    tpr = (np.arange(WBLK) - (WBLK - 1) / 2.0) * delta
    TE0 = tpr.astype(f32)[None, :]
    TE1 = a_psi * tpr * tpr

    def hi_lo(vals):
        hi = np.round(vals * 4.0) / 4.0
        lo = vals - hi
        return hi.astype(f16), lo.astype(f16)

    # BDE rhs rows: [TE1_hi tiled, TE1_lo tiled, then diag-ones pairs]
    BDE = np.zeros((2 * MAXNE + 2, MAXNE * WBLK), f16)
    te1_hi, te1_lo = hi_lo(TE1)
    for c in range(MAXNE):
        BDE[0, WBLK * c:WBLK * (c + 1)] = te1_hi
        BDE[1, WBLK * c:WBLK * (c + 1)] = te1_lo
        BDE[2 + 2 * c:4 + 2 * c, WBLK * c:WBLK * (c + 1)] = 1

    t_pad = np.zeros(T_GRID + 4, f64)
    t_pad[2:2 + T_GRID] = t64
    TCONV = np.zeros((16, T_GRID), f64)
    for o in range(5):
        TCONV += w[0][:, 0, o].astype(f64)[:, None] * t_pad[o:o + T_GRID][None, :]
    TCONV += bs[0].astype(f64)[:, None]

    W1f = np.zeros((2, 80), f16)      # [in=2, taps x out16]
    for o in range(5):
        W1f[:, 16 * o:16 * (o + 1)] = w[0][:, 1:3, o].T.astype(f16)
    W2f = np.zeros((16, 160), f16)
    for o in range(5):
        W2f[:, 32 * o:32 * (o + 1)] = w[1][:, :, o].T.astype(f16)
    W3f = np.zeros((128, 16), f16)    # taps 0-3 folded into partitions
    for o in range(4):
        W3f[32 * o:32 * (o + 1), :] = w[2][:, :, o].T.astype(f16)
    W3e = np.ascontiguousarray(w[2][:, :, 4].T).astype(f16)
    W4f = np.zeros((16, 10), f16)
    for o in range(5):
        W4f[:, 2 * o:2 * (o + 1)] = w[3][:, :, o].T.astype(f16)

    shared = {
        "TE0": TE0, "BDE": BDE,
        "TCONV": TCONV.astype(f32),
        "W1f": W1f, "W2f": W2f, "W3f": W3f, "W3e": W3e, "W4f": W4f,
        "B2": bs[1][:, None].copy(), "B3": bs[2][:, None].copy(),
        "ID2": np.eye(2, dtype=f16),
    }

    in_maps = []
    for core in range(NCORES):
        m = dict(shared)
        XCS = np.zeros((BLOC, 128, SE), f32)
        PHI = np.zeros((BLOC, 128, 2 * SE), f16)
        XCB = np.zeros((BLOC, 2 * MAXNE + 2, NBLK * 128), f16)
        XCB[:, 0:2, :] = 1
        XTP = np.zeros((BLOC, NBLK, TGTU), f32)
        XTQ2 = np.zeros((BLOC, 6, NBLK * 2 * TGTU), f16)
        TDS = np.zeros((BLOC, 128, SD), f32)
        TDB0 = np.zeros((BLOC, 6, NBLK * 128), f16)
        TDB0[:, 0:2, :] = 1
        TDB1 = np.zeros((BLOC, 6, NBLK * 128), f16)
        TDB1[:, 0:2, :] = 1
        for bb in range(BLOC):
            b = core * BLOC + bb
            base = 0
            for k in range(NBLK):
                ck = (t64[WBLK * k] + t64[WBLK * (k + 1) - 1]) / 2.0
                i0, i1 = eidx[b, k]
                nv = int(i1 - i0)
                ns = 128 * NCH_E[k]
                xv = np.zeros(ns, f64)
                xv[:nv] = xcs[b, i0:i1] - ck
                bias = np.full(ns, -60.0, f64)
                bias[:nv] = a_psi * xv[:nv] * xv[:nv]
                ph = np.zeros((ns, 2), f64)
                ph[:nv, 0] = os_psi
                ph[:nv, 1] = os_psi * ycs[b, i0:i1]
                for c in range(NCH_E[k]):
                    sl = slice(128 * c, 128 * (c + 1))
                    XCS[bb, :, base + c] = (-2.0 * a_psi * xv[sl]).astype(f32)
                    PHI[bb, :, 2 * (base + c)] = ph[sl, 0].astype(f16)
                    PHI[bb, :, 2 * (base + c) + 1] = ph[sl, 1].astype(f16)
                    hi, lo = hi_lo(bias[sl])
                    XCB[bb, 2 + 2 * c, 128 * k:128 * (k + 1)] = hi
                    XCB[bb, 3 + 2 * c, 128 * k:128 * (k + 1)] = lo
                base += NCH_E[k]
            base = 0
            for k in range(NBLK):
                gv = t64[128 * J0S[k]:128 * J1S[k]]
                cb = (gv[0] + gv[-1]) / 2.0
                i0, i1 = WBLK * k, WBLK * (k + 1)
                # window coverage check (chunks must span the band)
                assert xts[b, i0] - m_rho >= gv[0] - delta or J0S[k] == 0
                assert xts[b, i1 - 1] + m_rho <= gv[-1] + delta                     or J1S[k] == 16
                xv = xts[b, i0:i1] - cb
                XTP[bb, k, :] = xv.astype(f32)
                xq_hi, xq_lo = hi_lo(a_rho * xv * xv)
                k0 = 2 * TGTU * k
                for cc in range(2):
                    XTQ2[bb, 0, k0 + TGTU * cc:k0 + TGTU * (cc + 1)] = xq_hi
                    XTQ2[bb, 1, k0 + TGTU * cc:k0 + TGTU * (cc + 1)] = xq_lo
                    XTQ2[bb, 2 + 2 * cc:4 + 2 * cc,
                         k0 + TGTU * cc:k0 + TGTU * (cc + 1)] = 1
                tv = gv - cb
                for c in range(NCH_D[k]):
                    sl = slice(128 * c, 128 * (c + 1))
                    TDS[bb, :, base + c] = (-2.0 * a_rho * tv[sl]).astype(f32)
                    hi, lo = hi_lo(a_rho * tv[sl] * tv[sl])
                    half, cl = divmod(c, 2)
                    dst = TDB0 if half == 0 else TDB1
                    dst[bb, 2 + 2 * cl, 128 * k:128 * (k + 1)] = hi
                    dst[bb, 3 + 2 * cl, 128 * k:128 * (k + 1)] = lo
                base += NCH_D[k]
        m["XCS"] = XCS
        m["PHI"] = PHI
        m["XCB"] = XCB
        m["XTP"] = XTP
        m["XTQ2"] = XTQ2
        m["TDS"] = TDS
        m["TDB0"] = TDB0
        m["TDB1"] = TDB1
        in_maps.append(m)

    cfg = {
        "NCH_E": NCH_E, "NCH_D": NCH_D, "J0S": J0S, "TGTU": TGTU,
        "os_rho": float(os_rho), "b4_0": float(bs[3][0]),
        "b4_1": float(bs[3][1]), "stt_dve_frac": 1.0,
    }
    aux = {"perm_t": perm_t, "tsplit": tsplit, "TGTU": TGTU}
    return in_maps, cfg, aux


def kernel(**inputs):
    from concourse.bass_utils import run_bass_kernel_spmd

    in_maps, cfg, aux = make_inmaps(inputs)
    key = (tuple(cfg["NCH_E"]), tuple(cfg["NCH_D"]), tuple(cfg["J0S"]),
           cfg["TGTU"], cfg["os_rho"], cfg["b4_0"], cfg["b4_1"])
    if key not in _PROG_CACHE:
        _PROG_CACHE[key] = build_program(cfg)
    nc = _PROG_CACHE[key]

    res = run_bass_kernel_spmd(nc, in_maps, core_ids=list(range(NCORES)))
    outs = [np.asarray(res.results[i]["out"]) for i in range(NCORES)]
    packed = np.concatenate(outs, 0)  # [B, 2, N] in sorted-xt order
    out = np.zeros((B, N, 2), np.float32)
    for b in range(B):
        out[b, aux["perm_t"][b], 0] = packed[b, 0]
        out[b, aux["perm_t"][b], 1] = packed[b, 1]
    return out



# revision 6
# speedup vs baseline: 1.3929x; 1.3929x over previous
"""ConvCNP1d Trainium2 kernel, v3: all-PE RBF exponents + batch-fused convs.

Banded RBF via host-side sorting as in v2 (ls = ln2 over a 128-unit range
means entries vanish beyond |d| ~ 2.7, so both kernel matrices are banded
once xc / xt are sorted; output is un-sorted on the host at the end).

v3 changes vs v2:
- The full RBF exponent a*(x-t)^2 = a*t'^2 + a*x'^2 - 2a*x'*t' is built by
  ONE PE matmul per tile: the squared terms use hi/lo fp16 rank-1 rows as
  before, and the cross term -2a*x'*t' is three more hi/lo rank-1 rows
  (u_hi*t_hi + u_hi*t_lo + u_lo*t_hi; the dropped u_lo*t_lo is < 1e-3 in
  the exponent).  This deletes every per-chunk DVE scalar_tensor_tensor
  and the [128 x W] broadcast DMAs of t'/xt' that dominated the v2 head.
- The conv decoder runs batch-fused: per-core batches b0/b1 are processed
  in a single matmul per chunk via block-diagonal weights, with all taps
  folded into the partition dim (shifted stack copies at legal 32-aligned
  partition bases, conv3-style).  The t input channel of conv1 is affine
  in the grid index, so it collapses to two static hi/lo rows plus a bias
  and an exact 4-column edge correction added into PSUM.
- The h0/h1 epilogue runs folded: DMA gathers h rows into [8, 256] tiles
  (128-lane DVE work instead of single-partition ops), and DMA scatters
  h0 / h1*rec directly into the conv1 tap stack.
- Decoder K-tile generation (independent of the conv chain) is woven
  through the encoder and conv phases to keep PE/Scalar busy.
"""

import numpy as np

T_GRID = 2048
B = 16
N = 2048
NCORES = 8
BLOC = B // NCORES
NBLK = 8
WBLK = T_GRID // NBLK  # 256
ETH = 7.5              # exponent cutoff; kernel entries below e^-ETH dropped

_PROG_CACHE = {}


def build_program(cfg):
    import concourse.bacc as bacc
    import concourse.tile as tile
    from concourse import mybir

    f32 = mybir.dt.float32
    f16 = mybir.dt.float16
    AF = mybir.ActivationFunctionType
    OP = mybir.AluOpType

    NCH_E = cfg["NCH_E"]
    NCH_D = cfg["NCH_D"]
    J0S = cfg["J0S"]
    TGTU = cfg["TGTU"]
    SE = sum(NCH_E)
    MAXNE = max(NCH_E)
    MAXND = max(NCH_D)
    os_rho = cfg["os_rho"]
    b4_0 = cfg["b4_0"]
    b4_1 = cfg["b4_1"]
    RE = 2 + 5 * MAXNE           # encoder kgen rows
    RD = 12                      # decoder kgen rows (2 + 5*2 per half)
    TP = T_GRID + 8              # padded stack width (data at col j+4-o)
    assert MAXNE * WBLK <= 1024 and MAXND <= 4

    nc = bacc.Bacc(None, target_bir_lowering=False)

    BDEh = nc.declare_dram_parameter("BDE", [RE, MAXNE * WBLK], f16, isOutput=False)
    TDB0h = nc.declare_dram_parameter("TDB0", [RD, NBLK * 128], f16, isOutput=False)
    TDB1h = nc.declare_dram_parameter("TDB1", [RD, NBLK * 128], f16, isOutput=False)
    TROWh = nc.declare_dram_parameter("TROW", [2, T_GRID], f16, isOutput=False)
    W1h = nc.declare_dram_parameter("W1n", [26, 32], f16, isOutput=False)
    W2h = nc.declare_dram_parameter("W2n", [128, 64], f16, isOutput=False)
    W2eh = nc.declare_dram_parameter("W2e", [32, 64], f16, isOutput=False)
    W3h = nc.declare_dram_parameter("W3n", [128, 32], f16, isOutput=False)
    W3eh = nc.declare_dram_parameter("W3e", [64, 96], f16, isOutput=False)
    W4h = nc.declare_dram_parameter("W4n", [128, 4], f16, isOutput=False)
    W4eh = nc.declare_dram_parameter("W4e", [32, 4], f16, isOutput=False)
    C1h = nc.declare_dram_parameter("C1n", [32, 1], f32, isOutput=False)
    B2h = nc.declare_dram_parameter("B2n", [64, 1], f32, isOutput=False)
    B3h = nc.declare_dram_parameter("B3n", [32, 1], f32, isOutput=False)
    CRh = nc.declare_dram_parameter("CRn", [32, 4], f32, isOutput=False)
    ID4h = nc.declare_dram_parameter("ID4", [4, 4], f16, isOutput=False)
    XCBh = nc.declare_dram_parameter("XCB", [BLOC, RE, NBLK * 128], f16, isOutput=False)
    PHIh = nc.declare_dram_parameter("PHI", [BLOC, 128, 2 * SE], f16, isOutput=False)
    XTQh = nc.declare_dram_parameter("XTQ", [BLOC, RD, NBLK * 2 * TGTU], f16, isOutput=False)
    OUTh = nc.declare_dram_parameter("out", [BLOC, 2, NBLK * TGTU], f32, isOutput=True)

    with tile.TileContext(nc) as tc:
        with (
            tc.tile_pool(name="singles", bufs=1) as singles,
            tc.tile_pool(name="perb", bufs=2) as perb,
            tc.tile_pool(name="kpool", bufs=3) as kpool,
            tc.tile_pool(name="k2keep", bufs=1) as k2keep,
            tc.tile_pool(name="small", bufs=1) as small,
            tc.tile_pool(name="psE", bufs=2, space="PSUM") as psE,
            tc.tile_pool(name="psC", bufs=2, space="PSUM") as psC,
            tc.tile_pool(name="psH", bufs=2, space="PSUM") as psH,
        ):
            # ---- static loads, critical-path first ----
            BDE_sb = singles.tile([RE, MAXNE * WBLK], f16)
            nc.sync.dma_start(out=BDE_sb, in_=BDEh[:, :])
            st = [dict() for _ in range(BLOC)]

            def loads(b):
                s = st[b]
                s["XCB"] = perb.tile([RE, NBLK * 128], f16, tag="XCB", name="XCB_sb")
                nc.sync.dma_start(out=s["XCB"], in_=XCBh[b])
                s["PHI"] = perb.tile([128, 2 * SE], f16, tag="PHI", name="PHI_sb")
                nc.sync.dma_start(out=s["PHI"], in_=PHIh[b])
                s["XTQ"] = perb.tile([RD, NBLK * 2 * TGTU], f16, tag="XTQ", name="XTQ_sb")
                nc.sync.dma_start(out=s["XTQ"], in_=XTQh[b])
                s["h"] = perb.tile([2, T_GRID], f32, tag="h_sb", name="h_sb")
                s["hg0"] = perb.tile([8, WBLK], f32, tag="hg0", name="hg0")
                s["hg1"] = perb.tile([8, WBLK], f32, tag="hg1", name="hg1")
                s["rec"] = perb.tile([8, WBLK], f32, tag="rec", name="rec")
                s["h0f"] = perb.tile([8, WBLK], f16, tag="h0f", name="h0f")
                s["ratf"] = perb.tile([8, WBLK], f16, tag="ratf", name="ratf")
                s["fT"] = perb.tile([128, 2, 16], f16, tag="fT", name="fT")

            loads(0)
            TDB0_sb = singles.tile([RD, NBLK * 128], f16)
            nc.sync.dma_start(out=TDB0_sb, in_=TDB0h[:, :])
            TDB1_sb = singles.tile([RD, NBLK * 128], f16)
            nc.sync.dma_start(out=TDB1_sb, in_=TDB1h[:, :])
            loads(1)

            # conv stacks (shared by both batches; taps in partition blocks)
            C1S = singles.tile([26, TP], f16)   # rows: t_hi,t_lo, 5 taps x 4
            nc.vector.memset(C1S, 0.0)
            nc.sync.dma_start(out=C1S[0:2, 2:2 + T_GRID], in_=TROWh[:, :])
            F2 = singles.tile([128, TP], f16)   # 4 taps x (16ch x 2b)
            F3 = singles.tile([128, TP], f16)   # 2 taps x (32ch x 2b)
            F4 = singles.tile([128, TP], f16)   # 4 taps x (16ch x 2b)
            for F, blk in ((F2, 32), (F3, 64), (F4, 32)):
                nc.vector.memset(F[0:blk, 0:4], 0.0)
                nc.vector.memset(F[0:blk, 4 + T_GRID:TP], 0.0)
                for base in range(32, 128, 32):
                    nc.vector.memset(F[base:base + 32, 0:TP], 0.0)

            W1_sb = singles.tile([26, 32], f16)
            nc.sync.dma_start(out=W1_sb, in_=W1h[:, :])
            W2_sb = singles.tile([128, 64], f16)
            nc.sync.dma_start(out=W2_sb, in_=W2h[:, :])
            W2e_sb = singles.tile([32, 64], f16)
            nc.sync.dma_start(out=W2e_sb, in_=W2eh[:, :])
            W3_sb = singles.tile([128, 32], f16)
            nc.sync.dma_start(out=W3_sb, in_=W3h[:, :])
            W3e_sb = singles.tile([64, 96], f16)
            nc.sync.dma_start(out=W3e_sb, in_=W3eh[:, :])
            W4_sb = singles.tile([128, 4], f16)
            nc.sync.dma_start(out=W4_sb, in_=W4h[:, :])
            W4e_sb = singles.tile([32, 4], f16)
            nc.sync.dma_start(out=W4e_sb, in_=W4eh[:, :])
            C1_sb = singles.tile([32, 1], f32)
            nc.sync.dma_start(out=C1_sb, in_=C1h[:, :])
            B2_sb = singles.tile([64, 1], f32)
            nc.sync.dma_start(out=B2_sb, in_=B2h[:, :])
            B3_sb = singles.tile([32, 1], f32)
            nc.sync.dma_start(out=B3_sb, in_=B3h[:, :])
            CR_sb = singles.tile([32, 4], f32)
            nc.sync.dma_start(out=CR_sb, in_=CRh[:, :])
            ID4_sb = singles.tile([4, 4], f16)
            nc.sync.dma_start(out=ID4_sb, in_=ID4h[:, :])
            FRAW = singles.tile([4, T_GRID], f16)  # b0mu,b0sg,b1mu,b1sg

            def enc_block(b, k):
                s = st[b]
                nch = NCH_E[k]
                base = sum(NCH_E[:k])
                rows = 2 + 5 * nch
                tot = nch * WBLK
                eps = psE.tile([128, MAXNE * WBLK], f32, tag="E", name="E_ps")
                for (c0, c1) in ([(0, tot)] if tot <= 512 else [(0, 512), (512, tot)]):
                    nc.tensor.matmul(
                        eps[:, c0:c1],
                        s["XCB"][0:rows, 128 * k:128 * (k + 1)],
                        BDE_sb[0:rows, c0:c1],
                        start=True, stop=True,
                    )
                kt = kpool.tile([128, MAXNE * WBLK], f16, tag="K", name="K1t")
                nc.scalar.activation(out=kt[:, 0:tot], in_=eps[:, 0:tot], func=AF.Exp)
                hps = psH.tile([2, WBLK], f32, tag="hms", name="h_ps")
                for c in range(nch):
                    nc.tensor.matmul(
                        hps,
                        s["PHI"][:, 2 * (base + c):2 * (base + c) + 2],
                        kt[:, WBLK * c:WBLK * (c + 1)],
                        start=(c == 0), stop=(c == nch - 1),
                    )
                nc.vector.tensor_copy(s["h"][:, WBLK * k:WBLK * (k + 1)], hps)

            def dec_half(b, k, half):
                s = st[b]
                nch = min(2, NCH_D[k] - 2 * half)
                rows = 2 + 5 * nch
                tot = nch * TGTU
                if half == 0:
                    s[f"k2t_{k}"] = k2keep.tile(
                        [128, MAXND * TGTU], f16, tag=f"k2_{b}_{k}",
                        name=f"k2_{b}_{k}")
                tdb = TDB0_sb if half == 0 else TDB1_sb
                eps = psE.tile([128, MAXNE * WBLK], f32, tag="E", name="E_ps")
                nc.tensor.matmul(
                    eps[:, 0:tot],
                    tdb[0:rows, 128 * k:128 * (k + 1)],
                    s["XTQ"][0:rows, 2 * TGTU * k:2 * TGTU * k + tot],
                    start=True, stop=True,
                )
                nc.scalar.activation(
                    out=s[f"k2t_{k}"][:, 2 * half * TGTU:2 * half * TGTU + tot],
                    in_=eps[:, 0:tot], func=AF.Exp)

            def epilogue(b):
                # fold h into [8, 256] tiles so the reciprocal/ratio run on
                # many lanes; DMA scatters build the conv1 tap stack rows.
                s = st[b]
                nc.sync.dma_start(out=s["hg0"], in_=s["h"][0:1, :])
                nc.sync.dma_start(out=s["hg1"], in_=s["h"][1:2, :])
                nc.vector.reciprocal_approx_fast(s["rec"], s["hg0"])
                nc.vector.tensor_mul(s["ratf"], s["hg1"], s["rec"])
                nc.vector.tensor_copy(s["h0f"], s["hg0"])
                for o in range(5):
                    r = 2 + 4 * o + 2 * b
                    nc.sync.dma_start(
                        out=C1S[r:r + 1, 4 - o:4 - o + T_GRID], in_=s["h0f"])
                    nc.sync.dma_start(
                        out=C1S[r + 1:r + 2, 4 - o:4 - o + T_GRID], in_=s["ratf"])

            def conv_chunk(l, n):
                c0 = 512 * n
                if l == 0:
                    ps = psC.tile([32, 512], f32, tag="c", name="c_ps")
                    nc.tensor.matmul(ps, W1_sb, C1S[:, 2 + c0:2 + c0 + 512],
                                     start=True, stop=True)
                    if n == 0:
                        nc.vector.tensor_add(ps[:, 0:2], ps[:, 0:2], CR_sb[:, 0:2])
                    if n == 3:
                        nc.vector.tensor_add(ps[:, 510:512], ps[:, 510:512],
                                             CR_sb[:, 2:4])
                    nc.scalar.activation(out=F2[0:32, 4 + c0:4 + c0 + 512],
                                         in_=ps, func=AF.Relu, bias=C1_sb)
                elif l == 1:
                    ps = psC.tile([64, 512], f32, tag="c", name="c_ps")
                    nc.tensor.matmul(ps, W2_sb, F2[:, 2 + c0:2 + c0 + 512],
                                     start=True, stop=False)
                    nc.tensor.matmul(ps, W2e_sb, F2[0:32, 6 + c0:6 + c0 + 512],
                                     start=False, stop=True)
                    nc.scalar.activation(out=F3[0:64, 4 + c0:4 + c0 + 512],
                                         in_=ps, func=AF.Relu, bias=B2_sb)
                elif l == 2:
                    ps = psC.tile([32, 512], f32, tag="c", name="c_ps")
                    nc.tensor.matmul(ps, W3_sb, F3[:, 2 + c0:2 + c0 + 512],
                                     start=True, stop=False)
                    for o in (2, 3, 4):
                        nc.tensor.matmul(
                            ps, W3e_sb[:, 32 * (o - 2):32 * (o - 1)],
                            F3[0:64, 2 + c0 + o:2 + c0 + o + 512],
                            start=False, stop=(o == 4))
                    nc.scalar.activation(out=F4[0:32, 4 + c0:4 + c0 + 512],
                                         in_=ps, func=AF.Relu, bias=B3_sb)
                else:
                    ps = psC.tile([4, 512], f32, tag="c", name="c_ps")
                    nc.tensor.matmul(ps, W4_sb, F4[:, 2 + c0:2 + c0 + 512],
                                     start=True, stop=False)
                    nc.tensor.matmul(ps, W4e_sb, F4[0:32, 6 + c0:6 + c0 + 512],
                                     start=False, stop=True)
                    nc.vector.tensor_copy(FRAW[:, c0:c0 + 512], ps)

            def stack_shift(F, blk, o):
                # tap block o = base block shifted o columns left
                nc.vector.tensor_copy(
                    F[blk * o:blk * o + blk, 0:TP - o], F[0:blk, o:TP])

            def fchain(b):
                s = st[b]
                ftp = psC.tile([128, 64], f16, tag="c", name="ftp")
                for j in range(16):
                    nc.tensor.transpose(
                        ftp[:, 4 * j:4 * j + 4],
                        FRAW[:, 128 * j:128 * (j + 1)],
                        ID4_sb)
                mu = ftp[:, 2 * b::4]
                sg = ftp[:, 2 * b + 1::4]
                t1 = small.tile([128, 16], f32, tag="t1", name="t1")
                t4 = small.tile([128, 16], f32, tag="t4", name="t4")
                nc.scalar.activation(
                    out=s["fT"][:, 0, :], in_=mu, func=AF.Identity,
                    scale=float(os_rho), bias=float(os_rho * b4_0))
                nc.scalar.activation(out=t1, in_=sg, func=AF.Abs,
                                     bias=float(b4_1))
                nc.scalar.activation(out=t1, in_=t1, func=AF.Exp, scale=-1.0)
                nc.scalar.activation(out=t1, in_=t1, func=AF.Ln, bias=1.0)
                nc.scalar.activation(out=t4, in_=sg, func=AF.Relu,
                                     scale=float(os_rho),
                                     bias=float(os_rho * b4_1))
                nc.vector.scalar_tensor_tensor(
                    s["fT"][:, 1, :], t1, float(os_rho), t4, OP.mult, OP.add)

            def dec_mm(b, k):
                s = st[b]
                kt = s[f"k2t_{k}"]
                nch = NCH_D[k]
                msps = psH.tile([2, TGTU], f32, tag="hms", name="ms_ps")
                for c in range(nch):
                    nc.tensor.matmul(
                        msps,
                        s["fT"][:, :, J0S[k] + c],
                        kt[:, TGTU * c:TGTU * (c + 1)],
                        start=(c == 0), stop=(c == nch - 1),
                    )
                osl = small.tile([2, TGTU], f32, tag="osb", name="osb",
                                 bufs=3)
                nc.vector.tensor_copy(osl, msps)
                nc.sync.dma_start(
                    out=OUTh[b, :, TGTU * k:TGTU * (k + 1)], in_=osl)

            # ---------------- emission ----------------
            dec_units = [(b, k, h) for b in range(BLOC)
                         for k in range(NBLK)
                         for h in range(2) if 2 * h < NCH_D[k]]
            du = [0]

            def emit_dec(nu=1):
                for _ in range(nu):
                    if du[0] < len(dec_units):
                        b, k, h = dec_units[du[0]]
                        dec_half(b, k, h)
                        du[0] += 1

            for k in range(NBLK):
                enc_block(0, k)
                emit_dec(1)
            epilogue(0)
            for k in range(NBLK):
                enc_block(1, k)
                emit_dec(1)
            epilogue(1)

            for n in range(4):          # conv1
                conv_chunk(0, n)
                emit_dec(1)
            for o in (1, 2, 3):
                stack_shift(F2, 32, o)
            emit_dec(1)
            for n in range(4):          # conv2
                conv_chunk(1, n)
                emit_dec(1)
            stack_shift(F3, 64, 1)
            emit_dec(1)
            for n in range(4):          # conv3
                conv_chunk(2, n)
                emit_dec(1)
            for o in (1, 2, 3):
                stack_shift(F4, 32, o)
            emit_dec(1)
            for n in range(4):          # conv4
                conv_chunk(3, n)
                emit_dec(1)
            emit_dec(len(dec_units))    # drain any remainder

            fchain(0)
            for k in range(NBLK):
                dec_mm(0, k)
            fchain(1)
            for k in range(NBLK):
                dec_mm(1, k)

    nc.compile()
    return nc


def _hi_lo(vals):
    """Split into f16-exact hi (multiples of 1/16) + small f16 lo."""
    f16, f64 = np.float16, np.float64
    hi = (np.round(np.asarray(vals, f64) * 16.0) / 16.0).astype(f16)
    lo = (np.asarray(vals, f64) - hi.astype(f64)).astype(f16)
    return hi, lo


def make_inmaps(inputs):
    f32 = np.float32
    f16 = np.float16
    f64 = np.float64
    xc = np.asarray(inputs["xc"])[..., 0].astype(f32)
    yc = np.asarray(inputs["yc"])[..., 0].astype(f32)
    xt = np.asarray(inputs["xt"])[..., 0].astype(f32)
    ls_psi = f64(np.float32(inputs["ls_psi"]))
    os_psi = f64(np.float32(inputs["os_psi"]))
    ls_rho = f64(np.float32(inputs["ls_rho"]))
    os_rho = f64(np.float32(inputs["os_rho"]))
    w = [np.asarray(inputs[f"w{i}"]).astype(f32) for i in (1, 2, 3, 4)]
    bs = [np.asarray(inputs[f"b{i}"]).astype(f32) for i in (1, 2, 3, 4)]

    lower = np.minimum(xc.min(), xt.min())
    upper = np.maximum(xc.max(), xt.max())
    t64 = np.linspace(f64(lower), f64(upper), T_GRID)
    delta = (t64[-1] - t64[0]) / (T_GRID - 1)

    a_psi = -0.5 / (ls_psi * ls_psi)
    a_rho = -0.5 / (ls_rho * ls_rho)
    m_psi = np.sqrt(ETH / -a_psi)
    m_rho = np.sqrt(ETH / -a_rho)

    perm_c = np.argsort(xc, axis=1, kind="stable")
    xcs = np.take_along_axis(xc, perm_c, 1).astype(f64)
    ycs = np.take_along_axis(yc, perm_c, 1).astype(f64)
    perm_t = np.argsort(xt, axis=1, kind="stable")
    xts = np.take_along_axis(xt, perm_t, 1).astype(f64)

    # encoder windows
    eidx = np.zeros((B, NBLK, 2), np.int64)
    for k in range(NBLK):
        lo = t64[WBLK * k] - m_psi
        hi = t64[WBLK * (k + 1) - 1] + m_psi
        for b in range(B):
            eidx[b, k, 0] = np.searchsorted(xcs[b], lo)
            eidx[b, k, 1] = np.searchsorted(xcs[b], hi)
    ecnt = eidx[:, :, 1] - eidx[:, :, 0]
    NCH_E = [max(1, int(np.ceil(ecnt[:, k].max() / 128))) for k in range(NBLK)]
    assert max(NCH_E) <= 4, NCH_E

    # decoder grid-chunk windows per xt quantile-block (compile-time const)
    TGTU = WBLK
    J0S, J1S = [], []
    for k in range(NBLK):
        xmin = min(xts[b, WBLK * k] for b in range(B))
        xmax = max(xts[b, WBLK * (k + 1) - 1] for b in range(B))
        g0 = max(0, int(np.searchsorted(t64, xmin - m_rho)) - 1)
        g1 = min(T_GRID - 1, int(np.searchsorted(t64, xmax + m_rho)))
        J0S.append(g0 // 128)
        J1S.append(g1 // 128 + 1)
    NCH_D = [J1S[k] - J0S[k] for k in range(NBLK)]
    assert max(NCH_D) <= 4, NCH_D
    SE = sum(NCH_E)
    MAXNE = max(NCH_E)
    RE = 2 + 5 * MAXNE
    RD = 12

    tpr = (np.arange(WBLK) - (WBLK - 1) / 2.0) * delta
    te2_hi, te2_lo = _hi_lo(a_psi * tpr * tpr)
    th_hi, th_lo = _hi_lo(tpr)

    # BDE rows: te2 hi/lo tiled; per chunk [ind, ind, th, tl, th]
    BDE = np.zeros((RE, MAXNE * WBLK), f16)
    for c in range(MAXNE):
        sl = slice(WBLK * c, WBLK * (c + 1))
        BDE[0, sl] = te2_hi
        BDE[1, sl] = te2_lo
        BDE[2 + 5 * c, sl] = 1
        BDE[3 + 5 * c, sl] = 1
        BDE[4 + 5 * c, sl] = th_hi
        BDE[5 + 5 * c, sl] = th_lo
        BDE[6 + 5 * c, sl] = th_hi

    # decoder static LHS: per half, rows [1,1, then per chunk-in-half
    # [gb_hi, gb_lo, v_hi, v_hi, v_lo]] over grid partitions
    TDB = [np.zeros((RD, NBLK * 128), f16) for _ in range(2)]
    for k in range(NBLK):
        gv = t64[128 * J0S[k]:128 * J1S[k]]
        cb = (gv[0] + gv[-1]) / 2.0
        tv = gv - cb
        ksl = slice(128 * k, 128 * (k + 1))
        TDB[0][0:2, ksl] = 1
        TDB[1][0:2, ksl] = 1
        for c in range(NCH_D[k]):
            half, cc = divmod(c, 2)
            tvc = tv[128 * c:128 * (c + 1)]
            gb_hi, gb_lo = _hi_lo(a_rho * tvc * tvc)
            v_hi, v_lo = _hi_lo(-2.0 * a_rho * tvc)
            TDB[half][2 + 5 * cc, ksl] = gb_hi
            TDB[half][3 + 5 * cc, ksl] = gb_lo
            TDB[half][4 + 5 * cc, ksl] = v_hi
            TDB[half][5 + 5 * cc, ksl] = v_hi
            TDB[half][6 + 5 * cc, ksl] = v_lo

    # t-channel of conv1 is affine in t: two static hi/lo rows + bias +
    # exact edge correction for the reference's zero-padding of t.
    t_hi, t_lo = _hi_lo(t64)
    TROW = np.stack([t_hi, t_lo], 0)
    A1 = w[0][:, 0, :].astype(f64).sum(1)                   # [16]
    C1 = bs[0].astype(f64) + delta * (w[0][:, 0, :].astype(f64)
                                      * (np.arange(5) - 2)).sum(1)
    L, U = t64[0], t64[-1]
    CR = np.zeros((32, 4), f64)
    w10 = w[0][:, 0, :].astype(f64)
    for half in range(2):
        r = slice(16 * half, 16 * half + 16)
        CR[r, 0] = -w10[:, 0] * (L - 2 * delta) - w10[:, 1] * (L - delta)
        CR[r, 1] = -w10[:, 0] * (L - delta)
        CR[r, 2] = -w10[:, 4] * (U + delta)
        CR[r, 3] = -w10[:, 3] * (U + delta) - w10[:, 4] * (U + 2 * delta)

    # block-diagonal batched conv weights
    W1n = np.zeros((26, 32), f16)
    W1n[0, :] = np.tile(A1.astype(f16), 2)
    W1n[1, :] = np.tile(A1.astype(f16), 2)
    for o in range(5):
        for half in range(2):
            W1n[2 + 4 * o + 2 * half, 16 * half:16 * half + 16] = \
                w[0][:, 1, o].astype(f16)
            W1n[3 + 4 * o + 2 * half, 16 * half:16 * half + 16] = \
                w[0][:, 2, o].astype(f16)
    W2n = np.zeros((128, 64), f16)
    for o in range(4):
        for half in range(2):
            W2n[32 * o + 16 * half:32 * o + 16 * half + 16,
                32 * half:32 * half + 32] = w[1][:, :, o].T.astype(f16)
    W2e = np.zeros((32, 64), f16)
    for half in range(2):
        W2e[16 * half:16 * half + 16, 32 * half:32 * half + 32] = \
            w[1][:, :, 4].T.astype(f16)
    W3n = np.zeros((128, 32), f16)
    for o in range(2):
        for half in range(2):
            W3n[64 * o + 32 * half:64 * o + 32 * half + 32,
                16 * half:16 * half + 16] = w[2][:, :, o].T.astype(f16)
    W3e = np.zeros((64, 96), f16)
    for o in (2, 3, 4):
        for half in range(2):
            W3e[32 * half:32 * half + 32,
                32 * (o - 2) + 16 * half:32 * (o - 2) + 16 * half + 16] = \
                w[2][:, :, o].T.astype(f16)
    W4n = np.zeros((128, 4), f16)
    for o in range(4):
        for half in range(2):
            W4n[32 * o + 16 * half:32 * o + 16 * half + 16,
                2 * half:2 * half + 2] = w[3][:, :, o].T.astype(f16)
    W4e = np.zeros((32, 4), f16)
    for half in range(2):
        W4e[16 * half:16 * half + 16, 2 * half:2 * half + 2] = \
            w[3][:, :, 4].T.astype(f16)

    C1n = np.concatenate([C1, C1]).astype(f32)[:, None]
    B2n = np.concatenate([bs[1], bs[1]]).astype(f32)[:, None]
    B3n = np.concatenate([bs[2], bs[2]]).astype(f32)[:, None]

    shared = {
        "BDE": BDE, "TDB0": TDB[0], "TDB1": TDB[1], "TROW": TROW,
        "W1n": W1n, "W2n": W2n, "W2e": W2e, "W3n": W3n, "W3e": W3e,
        "W4n": W4n, "W4e": W4e,
        "C1n": C1n, "B2n": B2n, "B3n": B3n,
        "CRn": CR.astype(f32), "ID4": np.eye(4, dtype=f16),
    }

    in_maps = []
    for core in range(NCORES):
        m = dict(shared)
        XCB = np.zeros((BLOC, RE, NBLK * 128), f16)
        XCB[:, 0:2, :] = 1
        PHI = np.zeros((BLOC, 128, 2 * SE), f16)
        XTQ = np.zeros((BLOC, RD, NBLK * 2 * TGTU), f16)
        for bb in range(BLOC):
            b = core * BLOC + bb
            base = 0
            for k in range(NBLK):
                ck = (t64[WBLK * k] + t64[WBLK * (k + 1) - 1]) / 2.0
                i0, i1 = eidx[b, k]
                nv = int(i1 - i0)
                ns = 128 * NCH_E[k]
                xv = np.zeros(ns, f64)
                xv[:nv] = xcs[b, i0:i1] - ck
                bias = np.full(ns, -60.0, f64)
                bias[:nv] = a_psi * xv[:nv] * xv[:nv]
                uv = np.zeros(ns, f64)
                uv[:nv] = -2.0 * a_psi * xv[:nv]
                ph = np.zeros((ns, 2), f64)
                ph[:nv, 0] = os_psi
                ph[:nv, 1] = os_psi * ycs[b, i0:i1]
                ksl = slice(128 * k, 128 * (k + 1))
                for c in range(NCH_E[k]):
                    sl = slice(128 * c, 128 * (c + 1))
                    b_hi, b_lo = _hi_lo(bias[sl])
                    u_hi, u_lo = _hi_lo(uv[sl])
                    XCB[bb, 2 + 5 * c, ksl] = b_hi
                    XCB[bb, 3 + 5 * c, ksl] = b_lo
                    XCB[bb, 4 + 5 * c, ksl] = u_hi
                    XCB[bb, 5 + 5 * c, ksl] = u_hi
                    XCB[bb, 6 + 5 * c, ksl] = u_lo
                    PHI[bb, :, 2 * (base + c)] = ph[sl, 0].astype(f16)
                    PHI[bb, :, 2 * (base + c) + 1] = ph[sl, 1].astype(f16)
                base += NCH_E[k]
            for k in range(NBLK):
                gv = t64[128 * J0S[k]:128 * J1S[k]]
                cb = (gv[0] + gv[-1]) / 2.0
                i0, i1 = WBLK * k, WBLK * (k + 1)
                assert xts[b, i0] - m_rho >= gv[0] - delta or J0S[k] == 0
                assert xts[b, i1 - 1] + m_rho <= gv[-1] + delta \
                    or J1S[k] == 16
                xv = xts[b, i0:i1] - cb
                xb_hi, xb_lo = _hi_lo(a_rho * xv * xv)
                x_hi, x_lo = _hi_lo(xv)
                k0 = 2 * TGTU * k
                for cc in range(2):
                    csl = slice(k0 + TGTU * cc, k0 + TGTU * (cc + 1))
                    XTQ[bb, 0, csl] = xb_hi
                    XTQ[bb, 1, csl] = xb_lo
                    XTQ[bb, 2 + 5 * cc, csl] = 1
                    XTQ[bb, 3 + 5 * cc, csl] = 1
                    XTQ[bb, 4 + 5 * cc, csl] = x_hi
                    XTQ[bb, 5 + 5 * cc, csl] = x_lo
                    XTQ[bb, 6 + 5 * cc, csl] = x_hi
        m["XCB"] = XCB
        m["PHI"] = PHI
        m["XTQ"] = XTQ
        in_maps.append(m)

    cfg = {
        "NCH_E": NCH_E, "NCH_D": NCH_D, "J0S": J0S, "TGTU": TGTU,
        "os_rho": float(os_rho), "b4_0": float(bs[3][0]),
        "b4_1": float(bs[3][1]),
    }
    aux = {"perm_t": perm_t, "TGTU": TGTU}
    return in_maps, cfg, aux


def kernel(**inputs):
    from concourse.bass_utils import run_bass_kernel_spmd

    in_maps, cfg, aux = make_inmaps(inputs)
    key = (tuple(cfg["NCH_E"]), tuple(cfg["NCH_D"]), tuple(cfg["J0S"]),
           cfg["TGTU"], cfg["os_rho"], cfg["b4_0"], cfg["b4_1"])
    if key not in _PROG_CACHE:
        _PROG_CACHE[key] = build_program(cfg)
    nc = _PROG_CACHE[key]

    res = run_bass_kernel_spmd(nc, in_maps, core_ids=list(range(NCORES)))
    outs = [np.asarray(res.results[i]["out"]) for i in range(NCORES)]
    packed = np.concatenate(outs, 0)  # [B, 2, N] in sorted-xt order
    out = np.zeros((B, N, 2), np.float32)
    for b in range(B):
        out[b, aux["perm_t"][b], 0] = packed[b, 0]
        out[b, aux["perm_t"][b], 1] = packed[b, 1]
    return out


# revision 7
# speedup vs baseline: 1.4544x; 1.0441x over previous
"""ConvCNP1d Trainium2 kernel, v4.

Banded RBF via host-side sorting (ls = ln2 over a 128-unit range means
entries vanish beyond |d| ~ 2.7; output is un-sorted on the host).

Key structure (see v2/v3 history in git-less comments):
- RBF exponents a*(x-t)^2 are built entirely by one PE matmul per tile
  from hi/lo-split fp16 rank-1 rows (squared terms + cross term), then a
  single fused Exp emits the fp16 K tile.  No per-chunk DVE work.
- Encoder runs on 16 value-blocks of 128 grid points (narrow bands =>
  fewer padded (xc, t) pairs, and [128, <=512B] PSUM tiles so the eps
  pool can quadruple-buffer).  Decoder runs on 8 xt quantile-blocks of
  256 targets against fixed grid chunks.
- Conv decoder is batch-fused (block-diagonal weights process both
  per-core batches in one matmul) with taps folded into the partition
  dim via shifted stack copies at 32-aligned partition bases; tap 4 is
  a second matmul reading the base block at a column offset.  conv1's
  t channel is affine in the grid index: two static hi/lo t rows + a
  bias + an exact 4-column edge correction added into PSUM.
- h0/h1 epilogue folds h into [8, 256] tiles (DMA gather) so the
  reciprocal/ratio run wide, then DMA scatters into the conv1 stack.
- DMA descriptor generation on the sync engine (~0.6us per dma_start)
  is a hidden serializer: inputs are packed into 6 loads split across
  the two HWDGE queues (sync + scalar), outputs accumulate into one
  [2, 2048] tile per batch and leave in one DMA each.
"""

import numpy as np

T_GRID = 2048
B = 16
N = 2048
NCORES = 8
BLOC = B // NCORES
NBLK_E = 16
WBLK_E = T_GRID // NBLK_E   # 128
NBLK_D = 8
TGTU = T_GRID // NBLK_D     # 256
ETH = 7.5                   # exponent cutoff; entries below e^-ETH dropped
RD = 12                     # decoder kgen rows (2 + 5*2 per half)
TP = T_GRID + 8             # padded stack width (data at col j+4-o)

_PROG_CACHE = {}


def build_program(cfg):
    import concourse.bacc as bacc
    import concourse.tile as tile
    from concourse import mybir

    f32 = mybir.dt.float32
    f16 = mybir.dt.float16
    AF = mybir.ActivationFunctionType
    OP = mybir.AluOpType

    NCH_E = cfg["NCH_E"]
    NCH_D = cfg["NCH_D"]
    J0S = cfg["J0S"]
    os_rho = cfg["os_rho"]
    b4_0 = cfg["b4_0"]
    b4_1 = cfg["b4_1"]
    SE = sum(NCH_E)
    MAXNE = max(NCH_E)
    MAXND = max(NCH_D)
    RE = 2 + 5 * MAXNE
    BW = MAXNE * WBLK_E                      # BDE col width
    KGWC = BW + 2 * NBLK_D * 128             # KGW cols
    XBC = NBLK_E * 128 + NBLK_D * 2 * TGTU   # XB cols
    assert MAXNE * WBLK_E <= 512 and MAXND <= 4

    nc = bacc.Bacc(None, target_bir_lowering=False)

    KGWh = nc.declare_dram_parameter("KGW", [17, KGWC], f16, isOutput=False)
    XBh = nc.declare_dram_parameter("XB", [BLOC, 17, XBC], f16, isOutput=False)
    PHIh = nc.declare_dram_parameter("PHI", [BLOC, 128, 2 * SE], f16, isOutput=False)
    WALLh = nc.declare_dram_parameter("WALL", [128, 332], f16, isOutput=False)
    BALLh = nc.declare_dram_parameter("BALL", [64, 8], f32, isOutput=False)
    TROWh = nc.declare_dram_parameter("TROW", [2, T_GRID], f16, isOutput=False)
    OUTh = nc.declare_dram_parameter("out", [BLOC, 2, T_GRID], f32, isOutput=True)

    with tile.TileContext(nc) as tc:
        with (
            tc.tile_pool(name="singles", bufs=1) as singles,
            tc.tile_pool(name="perb", bufs=2) as perb,
            tc.tile_pool(name="kpool", bufs=4) as kpool,
            tc.tile_pool(name="k2keep", bufs=1) as k2keep,
            tc.tile_pool(name="small", bufs=1) as small,
            tc.tile_pool(name="psE", bufs=4, space="PSUM") as psE,
            tc.tile_pool(name="psC", bufs=2, space="PSUM") as psC,
            tc.tile_pool(name="psH", bufs=2, space="PSUM") as psH,
        ):
            # ---- loads: few big DMAs, split across the two HWDGE queues ----
            KGW = singles.tile([17, KGWC], f16)
            nc.sync.dma_start(out=KGW, in_=KGWh[:, :])
            st = [dict() for _ in range(BLOC)]
            for b in range(BLOC):
                s = st[b]
                s["XB"] = perb.tile([17, XBC], f16, tag="XB", name="XB_sb")
                s["PHI"] = perb.tile([128, 2 * SE], f16, tag="PHI", name="PHI_sb")
                s["h"] = perb.tile([2, T_GRID], f32, tag="h_sb", name="h_sb")
                s["hg0"] = perb.tile([8, TGTU], f32, tag="hg0", name="hg0")
                s["hg1"] = perb.tile([8, TGTU], f32, tag="hg1", name="hg1")
                s["rec"] = perb.tile([8, TGTU], f32, tag="rec", name="rec")
                s["h0f"] = perb.tile([8, TGTU], f16, tag="h0f", name="h0f")
                s["ratf"] = perb.tile([8, TGTU], f16, tag="ratf", name="ratf")
                s["fT"] = perb.tile([128, 2, 16], f16, tag="fT", name="fT")
                s["osl"] = perb.tile([2, T_GRID], f32, tag="osl", name="osl")
            nc.sync.dma_start(out=st[0]["XB"], in_=XBh[0])
            nc.sync.dma_start(out=st[0]["PHI"], in_=PHIh[0])
            nc.scalar.dma_start(out=st[1]["XB"], in_=XBh[1])
            nc.scalar.dma_start(out=st[1]["PHI"], in_=PHIh[1])
            WALL = singles.tile([128, 332], f16)
            nc.scalar.dma_start(out=WALL, in_=WALLh[:, :])
            BALL = singles.tile([64, 8], f32)
            nc.scalar.dma_start(out=BALL, in_=BALLh[:, :])

            def bde(rows, c0, c1):
                return KGW[0:rows, c0:c1]

            def tdb(half, rows, k):
                o = BW + 1024 * half
                return KGW[0:rows, o + 128 * k:o + 128 * (k + 1)]

            def xcb(b, rows, k):
                return st[b]["XB"][0:rows, 128 * k:128 * (k + 1)]

            def xtq(b, rows, k, tot):
                o = NBLK_E * 128 + 2 * TGTU * k
                return st[b]["XB"][0:rows, o:o + tot]

            W1n = WALL[0:102, 0:32]
            W1e = WALL[0:6, 32:64]
            W2n = WALL[0:128, 64:128]
            W2e = WALL[0:32, 128:192]
            W3n = WALL[0:128, 192:224]
            W3e = WALL[0:64, 224:320]
            W4n = WALL[0:128, 320:324]
            W4e = WALL[0:32, 324:328]
            ID4 = WALL[0:4, 328:332]
            B2a = BALL[0:64, 0:1]
            B3a = BALL[0:32, 1:2]
            C1a = BALL[0:32, 2:3]
            CRa = BALL[0:32, 3:7]

            # conv stacks (shared by both batches; taps in partition blocks)
            C1S = singles.tile([102, TP], f16)  # 4 blocks x [th,tl,4 data]
            nc.vector.memset(C1S, 0.0)
            nc.sync.dma_start(out=C1S[0:2, 2:2 + T_GRID], in_=TROWh[:, :])
            F2 = singles.tile([128, TP], f16)   # 4 taps x (16ch x 2b)
            F3 = singles.tile([128, TP], f16)   # 2 taps x (32ch x 2b)
            F4 = singles.tile([128, TP], f16)   # 4 taps x (16ch x 2b)
            for F, blk in ((F2, 32), (F3, 64), (F4, 32)):
                nc.vector.memset(F[0:blk, 0:4], 0.0)
                nc.vector.memset(F[0:blk, 4 + T_GRID:TP], 0.0)
            for F, blk, shifts in ((F2, 32, (1, 2, 3)), (F3, 64, (1,)),
                                   (F4, 32, (1, 2, 3))):
                for o in shifts:
                    nc.vector.memset(F[blk * o:blk * o + blk, TP - o:TP], 0.0)
            FRAW = singles.tile([4, T_GRID], f16)  # b0mu,b0sg,b1mu,b1sg

            def enc_block(b, k):
                s = st[b]
                nch = NCH_E[k]
                base = sum(NCH_E[:k])
                rows = 2 + 5 * nch
                tot = nch * WBLK_E
                eps = psE.tile([128, 512], f32, tag="E", name="E_ps")
                nc.tensor.matmul(eps[:, 0:tot], xcb(b, rows, k),
                                 bde(rows, 0, tot), start=True, stop=True)
                kt = kpool.tile([128, BW], f16, tag="K", name="K1t")
                nc.scalar.activation(out=kt[:, 0:tot], in_=eps[:, 0:tot],
                                     func=AF.Exp)
                hps = psH.tile([2, TGTU], f32, tag="hms", name="h_ps")
                for c in range(nch):
                    nc.tensor.matmul(
                        hps[:, 0:WBLK_E],
                        s["PHI"][:, 2 * (base + c):2 * (base + c) + 2],
                        kt[:, WBLK_E * c:WBLK_E * (c + 1)],
                        start=(c == 0), stop=(c == nch - 1),
                    )
                nc.vector.tensor_copy(
                    s["h"][:, WBLK_E * k:WBLK_E * (k + 1)], hps[:, 0:WBLK_E])

            def dec_half(b, k, half):
                s = st[b]
                nch = min(2, NCH_D[k] - 2 * half)
                rows = 2 + 5 * nch
                tot = nch * TGTU
                if half == 0:
                    s[f"k2t_{k}"] = k2keep.tile(
                        [128, MAXND * TGTU], f16, tag=f"k2_{b}_{k}",
                        name=f"k2_{b}_{k}")
                eps = psE.tile([128, 512], f32, tag="E", name="E_ps")
                nc.tensor.matmul(eps[:, 0:tot], tdb(half, rows, k),
                                 xtq(b, rows, k, tot), start=True, stop=True)
                nc.scalar.activation(
                    out=s[f"k2t_{k}"][:, 2 * half * TGTU:2 * half * TGTU + tot],
                    in_=eps[:, 0:tot], func=AF.Exp)

            def epilogue(b):
                s = st[b]
                nc.sync.dma_start(out=s["hg0"], in_=s["h"][0:1, :])
                nc.sync.dma_start(out=s["hg1"], in_=s["h"][1:2, :])
                nc.vector.reciprocal_approx_fast(s["rec"], s["hg0"])
                nc.vector.tensor_mul(s["ratf"], s["hg1"], s["rec"])
                nc.vector.tensor_copy(s["h0f"], s["hg0"])
                nc.sync.dma_start(
                    out=C1S[2 + 2 * b:3 + 2 * b, 4:4 + T_GRID], in_=s["h0f"])
                nc.sync.dma_start(
                    out=C1S[3 + 2 * b:4 + 2 * b, 4:4 + T_GRID], in_=s["ratf"])

            def conv_chunk(l, n):
                c0 = 512 * n
                if l == 0:
                    ps = psC.tile([32, 512], f32, tag="c", name="c_ps")
                    nc.tensor.matmul(ps, W1n, C1S[:, 2 + c0:2 + c0 + 512],
                                     start=True, stop=False)
                    nc.tensor.matmul(ps, W1e, C1S[0:6, 6 + c0:6 + c0 + 512],
                                     start=False, stop=True)
                    if n == 0:
                        nc.vector.tensor_add(ps[:, 0:2], ps[:, 0:2], CRa[:, 0:2])
                    if n == 3:
                        nc.vector.tensor_add(ps[:, 510:512], ps[:, 510:512],
                                             CRa[:, 2:4])
                    nc.scalar.activation(out=F2[0:32, 4 + c0:4 + c0 + 512],
                                         in_=ps, func=AF.Relu, bias=C1a)
                elif l == 1:
                    ps = psC.tile([64, 512], f32, tag="c", name="c_ps")
                    nc.tensor.matmul(ps, W2n, F2[:, 2 + c0:2 + c0 + 512],
                                     start=True, stop=False)
                    nc.tensor.matmul(ps, W2e, F2[0:32, 6 + c0:6 + c0 + 512],
                                     start=False, stop=True)
                    nc.scalar.activation(out=F3[0:64, 4 + c0:4 + c0 + 512],
                                         in_=ps, func=AF.Relu, bias=B2a)
                elif l == 2:
                    ps = psC.tile([32, 512], f32, tag="c", name="c_ps")
                    nc.tensor.matmul(ps, W3n, F3[:, 2 + c0:2 + c0 + 512],
                                     start=True, stop=False)
                    for o in (2, 3, 4):
                        nc.tensor.matmul(
                            ps, W3e[:, 32 * (o - 2):32 * (o - 1)],
                            F3[0:64, 2 + c0 + o:2 + c0 + o + 512],
                            start=False, stop=(o == 4))
                    nc.scalar.activation(out=F4[0:32, 4 + c0:4 + c0 + 512],
                                         in_=ps, func=AF.Relu, bias=B3a)
                else:
                    ps = psC.tile([4, 512], f32, tag="c", name="c_ps")
                    nc.tensor.matmul(ps, W4n, F4[:, 2 + c0:2 + c0 + 512],
                                     start=True, stop=False)
                    nc.tensor.matmul(ps, W4e, F4[0:32, 6 + c0:6 + c0 + 512],
                                     start=False, stop=True)
                    nc.vector.tensor_copy(FRAW[:, c0:c0 + 512], ps)

            def stack_shift(F, blk, o, rows):
                nc.vector.tensor_copy(
                    F[blk * o:blk * o + rows, 0:TP - o], F[0:rows, o:TP])

            def fchain(b):
                s = st[b]
                ftp = psC.tile([128, 64], f16, tag="c", name="ftp")
                for j in range(16):
                    nc.tensor.transpose(
                        ftp[:, 4 * j:4 * j + 4],
                        FRAW[:, 128 * j:128 * (j + 1)],
                        ID4)
                mu = ftp[:, 2 * b::4]
                sg = ftp[:, 2 * b + 1::4]
                t1 = small.tile([128, 16], f32, tag="t1", name="t1")
                t4 = small.tile([128, 16], f32, tag="t4", name="t4")
                nc.scalar.activation(
                    out=s["fT"][:, 0, :], in_=mu, func=AF.Identity,
                    scale=float(os_rho), bias=float(os_rho * b4_0))
                nc.scalar.activation(out=t1, in_=sg, func=AF.Abs,
                                     bias=float(b4_1))
                nc.scalar.activation(out=t1, in_=t1, func=AF.Exp, scale=-1.0)
                nc.scalar.activation(out=t1, in_=t1, func=AF.Ln, bias=1.0)
                nc.scalar.activation(out=t4, in_=sg, func=AF.Relu,
                                     scale=float(os_rho),
                                     bias=float(os_rho * b4_1))
                nc.vector.scalar_tensor_tensor(
                    s["fT"][:, 1, :], t1, float(os_rho), t4, OP.mult, OP.add)

            def dec_mm(b, k):
                s = st[b]
                kt = s[f"k2t_{k}"]
                nch = NCH_D[k]
                msps = psH.tile([2, TGTU], f32, tag="hms", name="ms_ps")
                for c in range(nch):
                    nc.tensor.matmul(
                        msps,
                        s["fT"][:, :, J0S[k] + c],
                        kt[:, TGTU * c:TGTU * (c + 1)],
                        start=(c == 0), stop=(c == nch - 1),
                    )
                nc.vector.tensor_copy(
                    s["osl"][:, TGTU * k:TGTU * (k + 1)], msps)
                if k == NBLK_D - 1:
                    nc.sync.dma_start(out=OUTh[b], in_=s["osl"])

            # ---------------- emission ----------------
            dec_units = [(b, k, h) for b in range(BLOC)
                         for k in range(NBLK_D)
                         for h in range(2) if 2 * h < NCH_D[k]]
            du = [0]

            def emit_dec(nu=1):
                for _ in range(nu):
                    if du[0] < len(dec_units):
                        b, k, h = dec_units[du[0]]
                        dec_half(b, k, h)
                        du[0] += 1

            for k in range(NBLK_E):
                enc_block(0, k)
                if k % 2 == 1:
                    emit_dec(1)
            epilogue(0)
            for k in range(NBLK_E):
                enc_block(1, k)
                emit_dec(1)
            epilogue(1)
            for o in (1, 2, 3):
                stack_shift(C1S, 32, o, 6)

            for n in range(4):          # conv1
                conv_chunk(0, n)
                emit_dec(1)
            for o in (1, 2, 3):
                stack_shift(F2, 32, o, 32)
            emit_dec(1)
            for n in range(4):          # conv2
                conv_chunk(1, n)
                emit_dec(1)
            stack_shift(F3, 64, 1, 64)
            emit_dec(1)
            for n in range(4):          # conv3
                conv_chunk(2, n)
                emit_dec(1)
            for o in (1, 2, 3):
                stack_shift(F4, 32, o, 32)
            emit_dec(1)
            for n in range(4):          # conv4
                conv_chunk(3, n)
                emit_dec(1)
            emit_dec(len(dec_units))    # drain any remainder

            fchain(0)
            for k in range(NBLK_D):
                dec_mm(0, k)
            fchain(1)
            for k in range(NBLK_D):
                dec_mm(1, k)

    nc.compile()
    return nc


def _hi_lo(vals):
    """Split into f16-exact hi (multiples of 1/16) + small f16 lo."""
    f16, f64 = np.float16, np.float64
    hi = (np.round(np.asarray(vals, f64) * 16.0) / 16.0).astype(f16)
    lo = (np.asarray(vals, f64) - hi.astype(f64)).astype(f16)
    return hi, lo


def make_inmaps(inputs):
    f32 = np.float32
    f16 = np.float16
    f64 = np.float64
    xc = np.asarray(inputs["xc"])[..., 0].astype(f32)
    yc = np.asarray(inputs["yc"])[..., 0].astype(f32)
    xt = np.asarray(inputs["xt"])[..., 0].astype(f32)
    ls_psi = f64(np.float32(inputs["ls_psi"]))
    os_psi = f64(np.float32(inputs["os_psi"]))
    ls_rho = f64(np.float32(inputs["ls_rho"]))
    os_rho = f64(np.float32(inputs["os_rho"]))
    w = [np.asarray(inputs[f"w{i}"]).astype(f32) for i in (1, 2, 3, 4)]
    bs = [np.asarray(inputs[f"b{i}"]).astype(f32) for i in (1, 2, 3, 4)]

    lower = np.minimum(xc.min(), xt.min())
    upper = np.maximum(xc.max(), xt.max())
    t64 = np.linspace(f64(lower), f64(upper), T_GRID)
    delta = (t64[-1] - t64[0]) / (T_GRID - 1)

    a_psi = -0.5 / (ls_psi * ls_psi)
    a_rho = -0.5 / (ls_rho * ls_rho)
    m_psi = np.sqrt(ETH / -a_psi)
    m_rho = np.sqrt(ETH / -a_rho)

    perm_c = np.argsort(xc, axis=1, kind="stable")
    xcs = np.take_along_axis(xc, perm_c, 1).astype(f64)
    ycs = np.take_along_axis(yc, perm_c, 1).astype(f64)
    perm_t = np.argsort(xt, axis=1, kind="stable")
    xts = np.take_along_axis(xt, perm_t, 1).astype(f64)

    # encoder windows (16 blocks of 128 grid points)
    eidx = np.zeros((B, NBLK_E, 2), np.int64)
    for k in range(NBLK_E):
        lo = t64[WBLK_E * k] - m_psi
        hi = t64[WBLK_E * (k + 1) - 1] + m_psi
        for b in range(B):
            eidx[b, k, 0] = np.searchsorted(xcs[b], lo)
            eidx[b, k, 1] = np.searchsorted(xcs[b], hi)
    ecnt = eidx[:, :, 1] - eidx[:, :, 0]
    NCH_E = [max(1, int(np.ceil(ecnt[:, k].max() / 128)))
             for k in range(NBLK_E)]
    assert max(NCH_E) <= 4, NCH_E

    # decoder grid-chunk windows per xt quantile-block
    J0S, J1S = [], []
    for k in range(NBLK_D):
        xmin = min(xts[b, TGTU * k] for b in range(B))
        xmax = max(xts[b, TGTU * (k + 1) - 1] for b in range(B))
        g0 = max(0, int(np.searchsorted(t64, xmin - m_rho)) - 1)
        g1 = min(T_GRID - 1, int(np.searchsorted(t64, xmax + m_rho)))
        J0S.append(g0 // 128)
        J1S.append(g1 // 128 + 1)
    NCH_D = [J1S[k] - J0S[k] for k in range(NBLK_D)]
    assert max(NCH_D) <= 4, NCH_D
    SE = sum(NCH_E)
    MAXNE = max(NCH_E)
    RE = 2 + 5 * MAXNE
    BW = MAXNE * WBLK_E
    KGWC = BW + 2 * NBLK_D * 128
    XBC = NBLK_E * 128 + NBLK_D * 2 * TGTU

    tpr = (np.arange(WBLK_E) - (WBLK_E - 1) / 2.0) * delta
    te2_hi, te2_lo = _hi_lo(a_psi * tpr * tpr)
    th_hi, th_lo = _hi_lo(tpr)

    # KGW: [BDE | TDB0 | TDB1]
    KGW = np.zeros((17, KGWC), f16)
    for c in range(MAXNE):
        sl = slice(WBLK_E * c, WBLK_E * (c + 1))
        KGW[0, sl] = te2_hi
        KGW[1, sl] = te2_lo
        KGW[2 + 5 * c, sl] = 1
        KGW[3 + 5 * c, sl] = 1
        KGW[4 + 5 * c, sl] = th_hi
        KGW[5 + 5 * c, sl] = th_lo
        KGW[6 + 5 * c, sl] = th_hi
    for k in range(NBLK_D):
        gv = t64[128 * J0S[k]:128 * J1S[k]]
        cb = (gv[0] + gv[-1]) / 2.0
        tv = gv - cb
        for half in range(2):
            o = BW + 1024 * half
            ksl = slice(o + 128 * k, o + 128 * (k + 1))
            KGW[0:2, ksl] = 1
        for c in range(NCH_D[k]):
            half, cc = divmod(c, 2)
            o = BW + 1024 * half
            ksl = slice(o + 128 * k, o + 128 * (k + 1))
            tvc = tv[128 * c:128 * (c + 1)]
            gb_hi, gb_lo = _hi_lo(a_rho * tvc * tvc)
            v_hi, v_lo = _hi_lo(-2.0 * a_rho * tvc)
            KGW[2 + 5 * cc, ksl] = gb_hi
            KGW[3 + 5 * cc, ksl] = gb_lo
            KGW[4 + 5 * cc, ksl] = v_hi
            KGW[5 + 5 * cc, ksl] = v_hi
            KGW[6 + 5 * cc, ksl] = v_lo

    # conv1 t channel: affine in t -> 2 static rows + bias + edge fix
    t_hi, t_lo = _hi_lo(t64)
    TROW = np.stack([t_hi, t_lo], 0)
    A1 = w[0][:, 0, :].astype(f64).sum(1)
    C1 = bs[0].astype(f64) + delta * (w[0][:, 0, :].astype(f64)
                                      * (np.arange(5) - 2)).sum(1)
    L, U = t64[0], t64[-1]
    CR = np.zeros((32, 4), f64)
    w10 = w[0][:, 0, :].astype(f64)
    for half in range(2):
        r = slice(16 * half, 16 * half + 16)
        CR[r, 0] = -w10[:, 0] * (L - 2 * delta) - w10[:, 1] * (L - delta)
        CR[r, 1] = -w10[:, 0] * (L - delta)
        CR[r, 2] = -w10[:, 4] * (U + delta)
        CR[r, 3] = -w10[:, 3] * (U + delta) - w10[:, 4] * (U + 2 * delta)

    # block-diagonal batched conv weights, packed into WALL [128, 332]
    W1n = np.zeros((128, 32), f16)
    for o in range(4):
        W1n[32 * o + 0, :] = np.tile(A1.astype(f16), 2) if o == 0 else 0
        W1n[32 * o + 1, :] = np.tile(A1.astype(f16), 2) if o == 0 else 0
        for half in range(2):
            W1n[32 * o + 2 + 2 * half, 16 * half:16 * half + 16] = \
                w[0][:, 1, o].astype(f16)
            W1n[32 * o + 3 + 2 * half, 16 * half:16 * half + 16] = \
                w[0][:, 2, o].astype(f16)
    W1e = np.zeros((6, 32), f16)
    for half in range(2):
        W1e[2 + 2 * half, 16 * half:16 * half + 16] = w[0][:, 1, 4].astype(f16)
        W1e[3 + 2 * half, 16 * half:16 * half + 16] = w[0][:, 2, 4].astype(f16)
    W2n = np.zeros((128, 64), f16)
    for o in range(4):
        for half in range(2):
            W2n[32 * o + 16 * half:32 * o + 16 * half + 16,
                32 * half:32 * half + 32] = w[1][:, :, o].T.astype(f16)
    W2e = np.zeros((32, 64), f16)
    for half in range(2):
        W2e[16 * half:16 * half + 16, 32 * half:32 * half + 32] = \
            w[1][:, :, 4].T.astype(f16)
    W3n = np.zeros((128, 32), f16)
    for o in range(2):
        for half in range(2):
            W3n[64 * o + 32 * half:64 * o + 32 * half + 32,
                16 * half:16 * half + 16] = w[2][:, :, o].T.astype(f16)
    W3e = np.zeros((64, 96), f16)
    for o in (2, 3, 4):
        for half in range(2):
            W3e[32 * half:32 * half + 32,
                32 * (o - 2) + 16 * half:32 * (o - 2) + 16 * half + 16] = \
                w[2][:, :, o].T.astype(f16)
    W4n = np.zeros((128, 4), f16)
    for o in range(4):
        for half in range(2):
            W4n[32 * o + 16 * half:32 * o + 16 * half + 16,
                2 * half:2 * half + 2] = w[3][:, :, o].T.astype(f16)
    W4e = np.zeros((32, 4), f16)
    for half in range(2):
        W4e[16 * half:16 * half + 16, 2 * half:2 * half + 2] = \
            w[3][:, :, 4].T.astype(f16)
    WALL = np.zeros((128, 332), f16)
    WALL[0:128, 0:32] = W1n
    WALL[0:6, 32:64] = W1e
    WALL[0:128, 64:128] = W2n
    WALL[0:32, 128:192] = W2e
    WALL[0:128, 192:224] = W3n
    WALL[0:64, 224:320] = W3e
    WALL[0:128, 320:324] = W4n
    WALL[0:32, 324:328] = W4e
    WALL[0:4, 328:332] = np.eye(4, dtype=f16)

    BALL = np.zeros((64, 8), f32)
    BALL[0:64, 0] = np.concatenate([bs[1], bs[1]])
    BALL[0:32, 1] = np.concatenate([bs[2], bs[2]])
    BALL[0:32, 2] = np.concatenate([C1, C1]).astype(f32)
    BALL[0:32, 3:7] = CR.astype(f32)

    shared = {"KGW": KGW, "WALL": WALL, "BALL": BALL, "TROW": TROW}

    in_maps = []
    for core in range(NCORES):
        m = dict(shared)
        XB = np.zeros((BLOC, 17, XBC), f16)
        PHI = np.zeros((BLOC, 128, 2 * SE), f16)
        for bb in range(BLOC):
            b = core * BLOC + bb
            XB[bb, 0:2, 0:NBLK_E * 128] = 1
            base = 0
            for k in range(NBLK_E):
                ck = (t64[WBLK_E * k] + t64[WBLK_E * (k + 1) - 1]) / 2.0
                i0, i1 = eidx[b, k]
                nv = int(i1 - i0)
                ns = 128 * NCH_E[k]
                xv = np.zeros(ns, f64)
                xv[:nv] = xcs[b, i0:i1] - ck
                bias = np.full(ns, -60.0, f64)
                bias[:nv] = a_psi * xv[:nv] * xv[:nv]
                uv = np.zeros(ns, f64)
                uv[:nv] = -2.0 * a_psi * xv[:nv]
                ph = np.zeros((ns, 2), f64)
                ph[:nv, 0] = os_psi
                ph[:nv, 1] = os_psi * ycs[b, i0:i1]
                ksl = slice(128 * k, 128 * (k + 1))
                for c in range(NCH_E[k]):
                    sl = slice(128 * c, 128 * (c + 1))
                    b_hi, b_lo = _hi_lo(bias[sl])
                    u_hi, u_lo = _hi_lo(uv[sl])
                    XB[bb, 2 + 5 * c, ksl] = b_hi
                    XB[bb, 3 + 5 * c, ksl] = b_lo
                    XB[bb, 4 + 5 * c, ksl] = u_hi
                    XB[bb, 5 + 5 * c, ksl] = u_hi
                    XB[bb, 6 + 5 * c, ksl] = u_lo
                    PHI[bb, :, 2 * (base + c)] = ph[sl, 0].astype(f16)
                    PHI[bb, :, 2 * (base + c) + 1] = ph[sl, 1].astype(f16)
                base += NCH_E[k]
            for k in range(NBLK_D):
                gv = t64[128 * J0S[k]:128 * J1S[k]]
                cb = (gv[0] + gv[-1]) / 2.0
                i0, i1 = TGTU * k, TGTU * (k + 1)
                assert xts[b, i0] - m_rho >= gv[0] - delta or J0S[k] == 0
                assert xts[b, i1 - 1] + m_rho <= gv[-1] + delta \
                    or J1S[k] == 16
                xv = xts[b, i0:i1] - cb
                xb_hi, xb_lo = _hi_lo(a_rho * xv * xv)
                x_hi, x_lo = _hi_lo(xv)
                k0 = NBLK_E * 128 + 2 * TGTU * k
                for cc in range(2):
                    csl = slice(k0 + TGTU * cc, k0 + TGTU * (cc + 1))
                    XB[bb, 0, csl] = xb_hi
                    XB[bb, 1, csl] = xb_lo
                    XB[bb, 2 + 5 * cc, csl] = 1
                    XB[bb, 3 + 5 * cc, csl] = 1
                    XB[bb, 4 + 5 * cc, csl] = x_hi
                    XB[bb, 5 + 5 * cc, csl] = x_lo
                    XB[bb, 6 + 5 * cc, csl] = x_hi
        m["XB"] = XB
        m["PHI"] = PHI
        in_maps.append(m)

    cfg = {
        "NCH_E": NCH_E, "NCH_D": NCH_D, "J0S": J0S,
        "os_rho": float(os_rho), "b4_0": float(bs[3][0]),
        "b4_1": float(bs[3][1]),
    }
    aux = {"perm_t": perm_t}
    return in_maps, cfg, aux


def kernel(**inputs):
    from concourse.bass_utils import run_bass_kernel_spmd

    in_maps, cfg, aux = make_inmaps(inputs)
    key = (tuple(cfg["NCH_E"]), tuple(cfg["NCH_D"]), tuple(cfg["J0S"]),
           cfg["os_rho"], cfg["b4_0"], cfg["b4_1"])
    if key not in _PROG_CACHE:
        _PROG_CACHE[key] = build_program(cfg)
    nc = _PROG_CACHE[key]

    res = run_bass_kernel_spmd(nc, in_maps, core_ids=list(range(NCORES)))
    outs = [np.asarray(res.results[i]["out"]) for i in range(NCORES)]
    packed = np.concatenate(outs, 0)  # [B, 2, N] in sorted-xt order
    out = np.zeros((B, N, 2), np.float32)
    for b in range(B):
        out[b, aux["perm_t"][b], 0] = packed[b, 0]
        out[b, aux["perm_t"][b], 1] = packed[b, 1]
    return out


# revision 17
# speedup vs baseline: 1.6882x; 1.1608x over previous
"""ConvCNP1d Trainium2 kernel, v4.

Banded RBF via host-side sorting (ls = ln2 over a 128-unit range means
entries vanish beyond |d| ~ 2.7; output is un-sorted on the host).

Key structure (see v2/v3 history in git-less comments):
- RBF exponents a*(x-t)^2 are built entirely by one PE matmul per tile
  from hi/lo-split fp16 rank-1 rows (squared terms + cross term), then a
  single fused Exp emits the fp16 K tile.  No per-chunk DVE work.
- Encoder runs on 16 value-blocks of 128 grid points (narrow bands =>
  fewer padded (xc, t) pairs, and [128, <=512B] PSUM tiles so the eps
  pool can quadruple-buffer).  Decoder runs on 8 xt quantile-blocks of
  256 targets against fixed grid chunks.
- Conv decoder is batch-fused (block-diagonal weights process both
  per-core batches in one matmul) with taps folded into the partition
  dim via shifted stack copies at 32-aligned partition bases; tap 4 is
  a second matmul reading the base block at a column offset.  conv1's
  t channel is affine in the grid index: two static hi/lo t rows + a
  bias + an exact 4-column edge correction added into PSUM.
- h0/h1 epilogue folds h into [8, 256] tiles (DMA gather) so the
  reciprocal/ratio run wide, then DMA scatters into the conv1 stack.
- DMA descriptor generation on the sync engine (~0.6us per dma_start)
  is a hidden serializer: inputs are packed into 6 loads split across
  the two HWDGE queues (sync + scalar), outputs accumulate into one
  [2, 2048] tile per batch and leave in one DMA each.
"""

import numpy as np

T_GRID = 2048
B = 16
N = 2048
NCORES = 8
BLOC = B // NCORES
NBLK_E = 16
WBLK_E = T_GRID // NBLK_E   # 128
NBLK_D = 8
TGTU = T_GRID // NBLK_D     # 256
ETH = 7.5                   # exponent cutoff; entries below e^-ETH dropped
RD = 12                     # decoder kgen rows (2 + 5*2 per half)
TP = T_GRID + 8             # padded stack width (data at col j+4-o)

_PROG_CACHE = {}


def build_program(cfg):
    import concourse.bacc as bacc
    import concourse.tile as tile
    from concourse import mybir

    f32 = mybir.dt.float32
    f16 = mybir.dt.float16
    AF = mybir.ActivationFunctionType
    OP = mybir.AluOpType

    NCH_E = cfg["NCH_E"]
    NCH_D = cfg["NCH_D"]
    J0S = cfg["J0S"]
    os_rho = cfg["os_rho"]
    b4_0 = cfg["b4_0"]
    b4_1 = cfg["b4_1"]
    SE = sum(NCH_E)
    MAXNE = max(NCH_E)
    MAXND = max(NCH_D)
    RE = 2 + 5 * MAXNE
    BW = MAXNE * WBLK_E                      # BDE col width
    KGWC = BW + 2 * NBLK_D * 128             # KGW cols
    XBC = NBLK_E * 128 + NBLK_D * 2 * TGTU   # XB cols
    assert MAXNE * WBLK_E <= 512 and MAXND <= 4

    nc = bacc.Bacc(None, target_bir_lowering=False)

    KGWh = nc.declare_dram_parameter("KGW", [17, KGWC], f16, isOutput=False)
    XBh = nc.declare_dram_parameter("XB", [BLOC, 17, XBC], f16, isOutput=False)
    PHIh = nc.declare_dram_parameter("PHI", [BLOC, 128, 2 * SE], f16, isOutput=False)
    WALLh = nc.declare_dram_parameter("WALL", [128, 332], f16, isOutput=False)
    BALLh = nc.declare_dram_parameter("BALL", [64, 8], f32, isOutput=False)
    TROWh = nc.declare_dram_parameter("TROW", [2, T_GRID], f16, isOutput=False)
    OUTh = nc.declare_dram_parameter("out", [BLOC, 2, T_GRID], f32, isOutput=True)

    with tile.TileContext(nc) as tc:
        with (
            tc.tile_pool(name="singles", bufs=1) as singles,
            tc.tile_pool(name="perb", bufs=2) as perb,
            tc.tile_pool(name="kpool", bufs=4) as kpool,
            tc.tile_pool(name="k2keep", bufs=1) as k2keep,
            tc.tile_pool(name="small", bufs=1) as small,
            tc.tile_pool(name="psE", bufs=4, space="PSUM") as psE,
            tc.tile_pool(name="psC", bufs=2, space="PSUM") as psC,
            tc.tile_pool(name="psH", bufs=2, space="PSUM") as psH,
        ):
            # ---- loads: split into queue-parallel pieces; a small first
            # piece covers the critical path (enc block 0 + PHI + TDB) ----
            KGW = singles.tile([17, KGWC], f16)
            st = [dict() for _ in range(BLOC)]
            for b in range(BLOC):
                s = st[b]
                s["XB"] = perb.tile([17, XBC], f16, tag="XB", name="XB_sb")
                s["PHI"] = perb.tile([128, 2 * SE], f16, tag="PHI", name="PHI_sb")
                s["h"] = perb.tile([2, T_GRID], f32, tag="h_sb", name="h_sb")
                s["hg0"] = perb.tile([8, TGTU], f32, tag="hg0", name="hg0")
                s["hg1"] = perb.tile([8, TGTU], f32, tag="hg1", name="hg1")
                s["rec"] = perb.tile([8, TGTU], f32, tag="rec", name="rec")
                s["h0f"] = perb.tile([8, TGTU], f16, tag="h0f", name="h0f")
                s["ratf"] = perb.tile([8, TGTU], f16, tag="ratf", name="ratf")
                s["fT"] = perb.tile([128, 2, 16], f16, tag="fT", name="fT")
                s["osl"] = perb.tile([2, T_GRID], f32, tag="osl", name="osl")
            NE = NBLK_E * 128
            nc.sync.dma_start(out=KGW[0:17, 0:BW], in_=KGWh[0:17, 0:BW])
            nc.sync.dma_start(out=st[0]["XB"][0:17, 0:512],
                              in_=XBh[0, 0:17, 0:512])
            nc.sync.dma_start(out=st[0]["PHI"], in_=PHIh[0])
            nc.sync.dma_start(out=KGW[0:12, BW:BW + 1024],
                              in_=KGWh[0:12, BW:BW + 1024])
            nc.sync.dma_start(out=st[0]["XB"][0:12, NE:NE + 2048],
                              in_=XBh[0, 0:12, NE:NE + 2048])
            nc.sync.dma_start(out=st[0]["XB"][0:17, 512:NE],
                              in_=XBh[0, 0:17, 512:NE])
            nc.sync.dma_start(out=KGW[0:12, BW + 1024:KGWC],
                              in_=KGWh[0:12, BW + 1024:KGWC])
            nc.sync.dma_start(out=st[0]["XB"][0:12, NE + 2048:XBC],
                              in_=XBh[0, 0:12, NE + 2048:XBC])
            nc.scalar.dma_start(out=st[1]["XB"][0:17, 0:NE],
                                in_=XBh[1, 0:17, 0:NE])
            nc.scalar.dma_start(out=st[1]["PHI"], in_=PHIh[1])
            WALL = singles.tile([128, 332], f16)
            nc.scalar.dma_start(out=WALL, in_=WALLh[:, :])
            nc.scalar.dma_start(out=st[1]["XB"][0:12, NE:NE + 2048],
                                in_=XBh[1, 0:12, NE:NE + 2048])
            nc.scalar.dma_start(out=st[1]["XB"][0:12, NE + 2048:XBC],
                                in_=XBh[1, 0:12, NE + 2048:XBC])
            BALL = singles.tile([64, 8], f32)
            nc.scalar.dma_start(out=BALL, in_=BALLh[:, :])

            def bde(rows, c0, c1):
                return KGW[0:rows, c0:c1]

            def tdb(half, rows, k):
                o = BW + 1024 * half
                return KGW[0:rows, o + 128 * k:o + 128 * (k + 1)]

            def xcb(b, rows, k):
                return st[b]["XB"][0:rows, 128 * k:128 * (k + 1)]

            def xtq(b, rows, k, tot):
                o = NBLK_E * 128 + 2 * TGTU * k
                return st[b]["XB"][0:rows, o:o + tot]

            W1n = WALL[0:100, 0:32]
            W1e = WALL[0:4, 32:64]
            W2n = WALL[0:128, 64:128]
            W2e = WALL[0:32, 128:192]
            W3n = WALL[0:128, 192:224]
            W3e = WALL[0:64, 224:320]
            W4n = WALL[0:128, 320:324]
            W4e = WALL[0:32, 324:328]
            ID4 = WALL[0:4, 328:332]
            B2a = BALL[0:64, 0:1]
            B3a = BALL[0:32, 1:2]
            C1a = BALL[0:32, 2:3]
            CRa = BALL[0:32, 3:7]

            # conv stacks (shared by both batches; taps in partition blocks;
            # block 0 rows 0-3 = data so shift copies read from base 0,
            # rows 4-5 of block 0 = the static affine t rows)
            C1S = singles.tile([100, TP], f16)
            nc.vector.memset(C1S, 0.0)
            nc.sync.dma_start(out=C1S[4:6, 2:2 + T_GRID], in_=TROWh[:, :])
            F2 = singles.tile([128, TP], f16)   # 4 taps x (16ch x 2b)
            F3 = singles.tile([128, TP], f16)   # 2 taps x (32ch x 2b)
            F4 = singles.tile([128, TP], f16)   # 4 taps x (16ch x 2b)
            for F, blk in ((F2, 32), (F3, 64), (F4, 32)):
                for o in range(128 // blk):
                    nc.vector.memset(F[blk * o:blk * o + blk, 0:4], 0.0)
                    nc.vector.memset(F[blk * o:blk * o + blk, T_GRID:TP], 0.0)
            FRAW = singles.tile([4, T_GRID], f16)  # b0mu,b0sg,b1mu,b1sg

            def enc_block(b, k):
                s = st[b]
                nch = NCH_E[k]
                base = sum(NCH_E[:k])
                rows = 2 + 5 * nch
                tot = nch * WBLK_E
                eps = psE.tile([128, 512], f32, tag="E", name="E_ps")
                nc.tensor.matmul(eps[:, 0:tot], xcb(b, rows, k),
                                 bde(rows, 0, tot), start=True, stop=True)
                kt = kpool.tile([128, BW], f16, tag="K", name="K1t")
                nc.scalar.activation(out=kt[:, 0:tot], in_=eps[:, 0:tot],
                                     func=AF.Exp)
                hps = psH.tile([2, TGTU], f32, tag="hms", name="h_ps")
                for c in range(nch):
                    nc.tensor.matmul(
                        hps[:, 0:WBLK_E],
                        s["PHI"][:, 2 * (base + c):2 * (base + c) + 2],
                        kt[:, WBLK_E * c:WBLK_E * (c + 1)],
                        start=(c == 0), stop=(c == nch - 1),
                    )
                nc.vector.tensor_copy(
                    s["h"][:, WBLK_E * k:WBLK_E * (k + 1)], hps[:, 0:WBLK_E])

            def dec_half(b, k, half):
                s = st[b]
                nch = min(2, NCH_D[k] - 2 * half)
                rows = 2 + 5 * nch
                tot = nch * TGTU
                if half == 0:
                    s[f"k2t_{k}"] = k2keep.tile(
                        [128, MAXND * TGTU], f16, tag=f"k2_{b}_{k}",
                        name=f"k2_{b}_{k}")
                eps = psE.tile([128, 512], f32, tag="E", name="E_ps")
                nc.tensor.matmul(eps[:, 0:tot], tdb(half, rows, k),
                                 xtq(b, rows, k, tot), start=True, stop=True)
                nc.scalar.activation(
                    out=s[f"k2t_{k}"][:, 2 * half * TGTU:2 * half * TGTU + tot],
                    in_=eps[:, 0:tot], func=AF.Exp)

            def epilogue(b):
                s = st[b]
                nc.sync.dma_start(out=s["hg0"], in_=s["h"][0:1, :])
                nc.sync.dma_start(out=s["hg1"], in_=s["h"][1:2, :])
                nc.vector.reciprocal_approx_fast(s["rec"], s["hg0"])
                nc.vector.tensor_mul(s["ratf"], s["hg1"], s["rec"])
                nc.vector.tensor_copy(s["h0f"], s["hg0"])
                nc.sync.dma_start(
                    out=C1S[2 * b:2 * b + 1, 4:4 + T_GRID], in_=s["h0f"])
                nc.sync.dma_start(
                    out=C1S[2 * b + 1:2 * b + 2, 4:4 + T_GRID], in_=s["ratf"])

            def conv_chunk(l, n):
                c0 = 512 * n
                if l == 0:
                    ps = psC.tile([32, 512], f32, tag="c", name="c_ps")
                    nc.tensor.matmul(ps, W1n, C1S[:, 2 + c0:2 + c0 + 512],
                                     start=True, stop=False)
                    nc.tensor.matmul(ps, W1e, C1S[0:4, 6 + c0:6 + c0 + 512],
                                     start=False, stop=True)
                    if n == 0:
                        nc.vector.tensor_add(ps[:, 0:2], ps[:, 0:2], CRa[:, 0:2])
                    if n == 3:
                        nc.vector.tensor_add(ps[:, 510:512], ps[:, 510:512],
                                             CRa[:, 2:4])
                    nc.scalar.activation(out=F2[0:32, 4 + c0:4 + c0 + 512],
                                         in_=ps, func=AF.Relu, bias=C1a)
                elif l == 1:
                    ps = psC.tile([64, 512], f32, tag="c", name="c_ps")
                    nc.tensor.matmul(ps, W2n, F2[:, 2 + c0:2 + c0 + 512],
                                     start=True, stop=False)
                    nc.tensor.matmul(ps, W2e, F2[0:32, 6 + c0:6 + c0 + 512],
                                     start=False, stop=True)
                    nc.scalar.activation(out=F3[0:64, 4 + c0:4 + c0 + 512],
                                         in_=ps, func=AF.Relu, bias=B2a)
                elif l == 2:
                    ps = psC.tile([32, 512], f32, tag="c", name="c_ps")
                    nc.tensor.matmul(ps, W3n, F3[:, 2 + c0:2 + c0 + 512],
                                     start=True, stop=False)
                    for o in (2, 3, 4):
                        nc.tensor.matmul(
                            ps, W3e[:, 32 * (o - 2):32 * (o - 1)],
                            F3[0:64, 2 + c0 + o:2 + c0 + o + 512],
                            start=False, stop=(o == 4))
                    nc.scalar.activation(out=F4[0:32, 4 + c0:4 + c0 + 512],
                                         in_=ps, func=AF.Relu, bias=B3a)
                else:
                    ps = psC.tile([4, 512], f32, tag="c", name="c_ps")
                    nc.tensor.matmul(ps, W4n, F4[:, 2 + c0:2 + c0 + 512],
                                     start=True, stop=False)
                    nc.tensor.matmul(ps, W4e, F4[0:32, 6 + c0:6 + c0 + 512],
                                     start=False, stop=True)
                    nc.vector.tensor_copy(FRAW[:, c0:c0 + 512], ps)

            def stack_shift(F, blk, rows, n):
                # per-chunk tap-block shifts so the next layer can start
                # before this layer's later chunks finish
                c0 = 512 * n
                for o in range(1, 128 // blk):
                    nc.vector.tensor_copy(
                        F[blk * o:blk * o + rows,
                          4 + c0 - o:4 + c0 + 512 - o],
                        F[0:rows, 4 + c0:4 + c0 + 512])

            def fchain(b):
                s = st[b]
                ftp = psC.tile([128, 64], f16, tag="c", name="ftp")
                for j in range(16):
                    nc.tensor.transpose(
                        ftp[:, 4 * j:4 * j + 4],
                        FRAW[:, 128 * j:128 * (j + 1)],
                        ID4)
                mu = ftp[:, 2 * b::4]
                sg = ftp[:, 2 * b + 1::4]
                t1 = small.tile([128, 16], f32, tag="t1", name="t1")
                t4 = small.tile([128, 16], f32, tag="t4", name="t4")
                nc.scalar.activation(
                    out=s["fT"][:, 0, :], in_=mu, func=AF.Identity,
                    scale=float(os_rho), bias=float(os_rho * b4_0))
                nc.scalar.activation(out=t1, in_=sg, func=AF.Abs,
                                     bias=float(b4_1))
                nc.scalar.activation(out=t1, in_=t1, func=AF.Exp, scale=-1.0)
                nc.scalar.activation(out=t1, in_=t1, func=AF.Ln, bias=1.0)
                nc.scalar.activation(out=t4, in_=sg, func=AF.Relu,
                                     scale=float(os_rho),
                                     bias=float(os_rho * b4_1))
                nc.vector.scalar_tensor_tensor(
                    s["fT"][:, 1, :], t1, float(os_rho), t4, OP.mult, OP.add)

            def dec_mm(b, k):
                s = st[b]
                kt = s[f"k2t_{k}"]
                nch = NCH_D[k]
                msps = psH.tile([2, TGTU], f32, tag="hms", name="ms_ps")
                for c in range(nch):
                    nc.tensor.matmul(
                        msps,
                        s["fT"][:, :, J0S[k] + c],
                        kt[:, TGTU * c:TGTU * (c + 1)],
                        start=(c == 0), stop=(c == nch - 1),
                    )
                nc.vector.tensor_copy(
                    s["osl"][:, TGTU * k:TGTU * (k + 1)], msps)
                if k == NBLK_D - 1:
                    nc.sync.dma_start(out=OUTh[b], in_=s["osl"])

            # ---------------- emission ----------------
            dec_units = [(b, k, h) for b in range(BLOC)
                         for k in range(NBLK_D)
                         for h in range(2) if 2 * h < NCH_D[k]]
            du = [0]

            def emit_dec(nu=1):
                for _ in range(nu):
                    if du[0] < len(dec_units):
                        b, k, h = dec_units[du[0]]
                        dec_half(b, k, h)
                        du[0] += 1

            for k in range(NBLK_E):
                enc_block(0, k)
                if k % 2 == 1:
                    emit_dec(1)
            epilogue(0)
            for k in range(NBLK_E):
                enc_block(1, k)
                emit_dec(1)
            epilogue(1)

            nexts = {0: (F2, 32, 32), 1: (F3, 64, 64), 2: (F4, 32, 32)}
            for n in range(4):
                stack_shift(C1S, 32, 4, n)
            for l in range(4):
                for n in range(4):
                    conv_chunk(l, n)
                    if l < 3 and n >= 1:
                        stack_shift(*nexts[l][:2], nexts[l][2], n - 1)
                    emit_dec(1)
                if l < 3:
                    stack_shift(*nexts[l][:2], nexts[l][2], 3)
            emit_dec(len(dec_units))    # drain any remainder

            fchain(0)
            fchain(1)
            for k in range(NBLK_D):
                dec_mm(0, k)
                dec_mm(1, k)

    nc.compile()
    return nc


def _hi_lo(vals):
    """Split into f16-exact hi (multiples of 1/16) + small f16 lo."""
    f16, f64 = np.float16, np.float64
    hi = (np.round(np.asarray(vals, f64) * 16.0) / 16.0).astype(f16)
    lo = (np.asarray(vals, f64) - hi.astype(f64)).astype(f16)
    return hi, lo


def make_inmaps(inputs):
    f32 = np.float32
    f16 = np.float16
    f64 = np.float64
    xc = np.asarray(inputs["xc"])[..., 0].astype(f32)
    yc = np.asarray(inputs["yc"])[..., 0].astype(f32)
    xt = np.asarray(inputs["xt"])[..., 0].astype(f32)
    ls_psi = f64(np.float32(inputs["ls_psi"]))
    os_psi = f64(np.float32(inputs["os_psi"]))
    ls_rho = f64(np.float32(inputs["ls_rho"]))
    os_rho = f64(np.float32(inputs["os_rho"]))
    w = [np.asarray(inputs[f"w{i}"]).astype(f32) for i in (1, 2, 3, 4)]
    bs = [np.asarray(inputs[f"b{i}"]).astype(f32) for i in (1, 2, 3, 4)]

    lower = np.minimum(xc.min(), xt.min())
    upper = np.maximum(xc.max(), xt.max())
    t64 = np.linspace(f64(lower), f64(upper), T_GRID)
    delta = (t64[-1] - t64[0]) / (T_GRID - 1)

    a_psi = -0.5 / (ls_psi * ls_psi)
    a_rho = -0.5 / (ls_rho * ls_rho)
    m_psi = np.sqrt(ETH / -a_psi)
    m_rho = np.sqrt(ETH / -a_rho)

    perm_c = np.argsort(xc, axis=1, kind="stable")
    xcs = np.take_along_axis(xc, perm_c, 1).astype(f64)
    ycs = np.take_along_axis(yc, perm_c, 1).astype(f64)
    perm_t = np.argsort(xt, axis=1, kind="stable")
    xts = np.take_along_axis(xt, perm_t, 1).astype(f64)

    # encoder windows (16 blocks of 128 grid points)
    eidx = np.zeros((B, NBLK_E, 2), np.int64)
    for k in range(NBLK_E):
        lo = t64[WBLK_E * k] - m_psi
        hi = t64[WBLK_E * (k + 1) - 1] + m_psi
        for b in range(B):
            eidx[b, k, 0] = np.searchsorted(xcs[b], lo)
            eidx[b, k, 1] = np.searchsorted(xcs[b], hi)
    ecnt = eidx[:, :, 1] - eidx[:, :, 0]
    NCH_E = [max(1, int(np.ceil(ecnt[:, k].max() / 128)))
             for k in range(NBLK_E)]
    assert max(NCH_E) <= 4, NCH_E

    # decoder grid-chunk windows per xt quantile-block
    J0S, J1S = [], []
    for k in range(NBLK_D):
        xmin = min(xts[b, TGTU * k] for b in range(B))
        xmax = max(xts[b, TGTU * (k + 1) - 1] for b in range(B))
        g0 = max(0, int(np.searchsorted(t64, xmin - m_rho)) - 1)
        g1 = min(T_GRID - 1, int(np.searchsorted(t64, xmax + m_rho)))
        J0S.append(g0 // 128)
        J1S.append(g1 // 128 + 1)
    NCH_D = [J1S[k] - J0S[k] for k in range(NBLK_D)]
    assert max(NCH_D) <= 4, NCH_D
    SE = sum(NCH_E)
    MAXNE = max(NCH_E)
    RE = 2 + 5 * MAXNE
    BW = MAXNE * WBLK_E
    KGWC = BW + 2 * NBLK_D * 128
    XBC = NBLK_E * 128 + NBLK_D * 2 * TGTU

    tpr = (np.arange(WBLK_E) - (WBLK_E - 1) / 2.0) * delta
    te2_hi, te2_lo = _hi_lo(a_psi * tpr * tpr)
    th_hi, th_lo = _hi_lo(tpr)

    # KGW: [BDE | TDB0 | TDB1]
    KGW = np.zeros((17, KGWC), f16)
    for c in range(MAXNE):
        sl = slice(WBLK_E * c, WBLK_E * (c + 1))
        KGW[0, sl] = te2_hi
        KGW[1, sl] = te2_lo
        KGW[2 + 5 * c, sl] = 1
        KGW[3 + 5 * c, sl] = 1
        KGW[4 + 5 * c, sl] = th_hi
        KGW[5 + 5 * c, sl] = th_lo
        KGW[6 + 5 * c, sl] = th_hi
    for k in range(NBLK_D):
        gv = t64[128 * J0S[k]:128 * J1S[k]]
        cb = (gv[0] + gv[-1]) / 2.0
        tv = gv - cb
        for half in range(2):
            o = BW + 1024 * half
            ksl = slice(o + 128 * k, o + 128 * (k + 1))
            KGW[0:2, ksl] = 1
        for c in range(NCH_D[k]):
            half, cc = divmod(c, 2)
            o = BW + 1024 * half
            ksl = slice(o + 128 * k, o + 128 * (k + 1))
            tvc = tv[128 * c:128 * (c + 1)]
            gb_hi, gb_lo = _hi_lo(a_rho * tvc * tvc)
            v_hi, v_lo = _hi_lo(-2.0 * a_rho * tvc)
            KGW[2 + 5 * cc, ksl] = gb_hi
            KGW[3 + 5 * cc, ksl] = gb_lo
            KGW[4 + 5 * cc, ksl] = v_hi
            KGW[5 + 5 * cc, ksl] = v_hi
            KGW[6 + 5 * cc, ksl] = v_lo

    # conv1 t channel: affine in t -> 2 static rows + bias + edge fix
    t_hi, t_lo = _hi_lo(t64)
    TROW = np.stack([t_hi, t_lo], 0)
    A1 = w[0][:, 0, :].astype(f64).sum(1)
    C1 = bs[0].astype(f64) + delta * (w[0][:, 0, :].astype(f64)
                                      * (np.arange(5) - 2)).sum(1)
    L, U = t64[0], t64[-1]
    CR = np.zeros((32, 4), f64)
    w10 = w[0][:, 0, :].astype(f64)
    for half in range(2):
        r = slice(16 * half, 16 * half + 16)
        CR[r, 0] = -w10[:, 0] * (L - 2 * delta) - w10[:, 1] * (L - delta)
        CR[r, 1] = -w10[:, 0] * (L - delta)
        CR[r, 2] = -w10[:, 4] * (U + delta)
        CR[r, 3] = -w10[:, 3] * (U + delta) - w10[:, 4] * (U + 2 * delta)

    # block-diagonal batched conv weights, packed into WALL [128, 332]
    W1n = np.zeros((128, 32), f16)
    W1n[4, :] = np.tile(A1.astype(f16), 2)
    W1n[5, :] = np.tile(A1.astype(f16), 2)
    for o in range(4):
        for half in range(2):
            W1n[32 * o + 2 * half, 16 * half:16 * half + 16] = \
                w[0][:, 1, o].astype(f16)
            W1n[32 * o + 1 + 2 * half, 16 * half:16 * half + 16] = \
                w[0][:, 2, o].astype(f16)
    W1e = np.zeros((4, 32), f16)
    for half in range(2):
        W1e[2 * half, 16 * half:16 * half + 16] = w[0][:, 1, 4].astype(f16)
        W1e[1 + 2 * half, 16 * half:16 * half + 16] = w[0][:, 2, 4].astype(f16)
    W2n = np.zeros((128, 64), f16)
    for o in range(4):
        for half in range(2):
            W2n[32 * o + 16 * half:32 * o + 16 * half + 16,
                32 * half:32 * half + 32] = w[1][:, :, o].T.astype(f16)
    W2e = np.zeros((32, 64), f16)
    for half in range(2):
        W2e[16 * half:16 * half + 16, 32 * half:32 * half + 32] = \
            w[1][:, :, 4].T.astype(f16)
    W3n = np.zeros((128, 32), f16)
    for o in range(2):
        for half in range(2):
            W3n[64 * o + 32 * half:64 * o + 32 * half + 32,
                16 * half:16 * half + 16] = w[2][:, :, o].T.astype(f16)
    W3e = np.zeros((64, 96), f16)
    for o in (2, 3, 4):
        for half in range(2):
            W3e[32 * half:32 * half + 32,
                32 * (o - 2) + 16 * half:32 * (o - 2) + 16 * half + 16] = \
                w[2][:, :, o].T.astype(f16)
    W4n = np.zeros((128, 4), f16)
    for o in range(4):
        for half in range(2):
            W4n[32 * o + 16 * half:32 * o + 16 * half + 16,
                2 * half:2 * half + 2] = w[3][:, :, o].T.astype(f16)
    W4e = np.zeros((32, 4), f16)
    for half in range(2):
        W4e[16 * half:16 * half + 16, 2 * half:2 * half + 2] = \
            w[3][:, :, 4].T.astype(f16)
    WALL = np.zeros((128, 332), f16)
    WALL[0:128, 0:32] = W1n
    WALL[0:4, 32:64] = W1e
    WALL[0:128, 64:128] = W2n
    WALL[0:32, 128:192] = W2e
    WALL[0:128, 192:224] = W3n
    WALL[0:64, 224:320] = W3e
    WALL[0:128, 320:324] = W4n
    WALL[0:32, 324:328] = W4e
    WALL[0:4, 328:332] = np.eye(4, dtype=f16)

    BALL = np.zeros((64, 8), f32)
    BALL[0:64, 0] = np.concatenate([bs[1], bs[1]])
    BALL[0:32, 1] = np.concatenate([bs[2], bs[2]])
    BALL[0:32, 2] = np.concatenate([C1, C1]).astype(f32)
    BALL[0:32, 3:7] = CR.astype(f32)

    shared = {"KGW": KGW, "WALL": WALL, "BALL": BALL, "TROW": TROW}

    in_maps = []
    for core in range(NCORES):
        m = dict(shared)
        XB = np.zeros((BLOC, 17, XBC), f16)
        PHI = np.zeros((BLOC, 128, 2 * SE), f16)
        for bb in range(BLOC):
            b = core * BLOC + bb
            XB[bb, 0:2, 0:NBLK_E * 128] = 1
            base = 0
            for k in range(NBLK_E):
                ck = (t64[WBLK_E * k] + t64[WBLK_E * (k + 1) - 1]) / 2.0
                i0, i1 = eidx[b, k]
                nv = int(i1 - i0)
                ns = 128 * NCH_E[k]
                xv = np.zeros(ns, f64)
                xv[:nv] = xcs[b, i0:i1] - ck
                bias = np.full(ns, -60.0, f64)
                bias[:nv] = a_psi * xv[:nv] * xv[:nv]
                uv = np.zeros(ns, f64)
                uv[:nv] = -2.0 * a_psi * xv[:nv]
                ph = np.zeros((ns, 2), f64)
                ph[:nv, 0] = os_psi
                ph[:nv, 1] = os_psi * ycs[b, i0:i1]
                ksl = slice(128 * k, 128 * (k + 1))
                for c in range(NCH_E[k]):
                    sl = slice(128 * c, 128 * (c + 1))
                    b_hi, b_lo = _hi_lo(bias[sl])
                    u_hi, u_lo = _hi_lo(uv[sl])
                    XB[bb, 2 + 5 * c, ksl] = b_hi
                    XB[bb, 3 + 5 * c, ksl] = b_lo
                    XB[bb, 4 + 5 * c, ksl] = u_hi
                    XB[bb, 5 + 5 * c, ksl] = u_hi
                    XB[bb, 6 + 5 * c, ksl] = u_lo
                    PHI[bb, :, 2 * (base + c)] = ph[sl, 0].astype(f16)
                    PHI[bb, :, 2 * (base + c) + 1] = ph[sl, 1].astype(f16)
                base += NCH_E[k]
            for k in range(NBLK_D):
                gv = t64[128 * J0S[k]:128 * J1S[k]]
                cb = (gv[0] + gv[-1]) / 2.0
                i0, i1 = TGTU * k, TGTU * (k + 1)
                assert xts[b, i0] - m_rho >= gv[0] - delta or J0S[k] == 0
                assert xts[b, i1 - 1] + m_rho <= gv[-1] + delta \
                    or J1S[k] == 16
                xv = xts[b, i0:i1] - cb
                xb_hi, xb_lo = _hi_lo(a_rho * xv * xv)
                x_hi, x_lo = _hi_lo(xv)
                k0 = NBLK_E * 128 + 2 * TGTU * k
                for cc in range(2):
                    csl = slice(k0 + TGTU * cc, k0 + TGTU * (cc + 1))
                    XB[bb, 0, csl] = xb_hi
                    XB[bb, 1, csl] = xb_lo
                    XB[bb, 2 + 5 * cc, csl] = 1
                    XB[bb, 3 + 5 * cc, csl] = 1
                    XB[bb, 4 + 5 * cc, csl] = x_hi
                    XB[bb, 5 + 5 * cc, csl] = x_lo
                    XB[bb, 6 + 5 * cc, csl] = x_hi
        m["XB"] = XB
        m["PHI"] = PHI
        in_maps.append(m)

    cfg = {
        "NCH_E": NCH_E, "NCH_D": NCH_D, "J0S": J0S,
        "os_rho": float(os_rho), "b4_0": float(bs[3][0]),
        "b4_1": float(bs[3][1]),
    }
    aux = {"perm_t": perm_t}
    return in_maps, cfg, aux


def kernel(**inputs):
    from concourse.bass_utils import run_bass_kernel_spmd

    in_maps, cfg, aux = make_inmaps(inputs)
    key = (tuple(cfg["NCH_E"]), tuple(cfg["NCH_D"]), tuple(cfg["J0S"]),
           cfg["os_rho"], cfg["b4_0"], cfg["b4_1"])
    if key not in _PROG_CACHE:
        _PROG_CACHE[key] = build_program(cfg)
    nc = _PROG_CACHE[key]

    res = run_bass_kernel_spmd(nc, in_maps, core_ids=list(range(NCORES)))
    outs = [np.asarray(res.results[i]["out"]) for i in range(NCORES)]
    packed = np.concatenate(outs, 0)  # [B, 2, N] in sorted-xt order
    out = np.zeros((B, N, 2), np.float32)
    for b in range(B):
        out[b, aux["perm_t"][b], 0] = packed[b, 0]
        out[b, aux["perm_t"][b], 1] = packed[b, 1]
    return out


# revision 21
# speedup vs baseline: 1.7134x; 1.0149x over previous
"""ConvCNP1d Trainium2 kernel, v4.

Banded RBF via host-side sorting (ls = ln2 over a 128-unit range means
entries vanish beyond |d| ~ 2.7; output is un-sorted on the host).

Key structure (see v2/v3 history in git-less comments):
- RBF exponents a*(x-t)^2 are built entirely by one PE matmul per tile
  from hi/lo-split fp16 rank-1 rows (squared terms + cross term), then a
  single fused Exp emits the fp16 K tile.  No per-chunk DVE work.
- Encoder runs on 16 value-blocks of 128 grid points (narrow bands =>
  fewer padded (xc, t) pairs, and [128, <=512B] PSUM tiles so the eps
  pool can quadruple-buffer).  Decoder runs on 8 xt quantile-blocks of
  256 targets against fixed grid chunks.
- Conv decoder is batch-fused (block-diagonal weights process both
  per-core batches in one matmul) with taps folded into the partition
  dim via shifted stack copies at 32-aligned partition bases; tap 4 is
  a second matmul reading the base block at a column offset.  conv1's
  t channel is affine in the grid index: two static hi/lo t rows + a
  bias + an exact 4-column edge correction added into PSUM.
- h0/h1 epilogue folds h into [8, 256] tiles (DMA gather) so the
  reciprocal/ratio run wide, then DMA scatters into the conv1 stack.
- DMA descriptor generation on the sync engine (~0.6us per dma_start)
  is a hidden serializer: inputs are packed into 6 loads split across
  the two HWDGE queues (sync + scalar), outputs accumulate into one
  [2, 2048] tile per batch and leave in one DMA each.
"""

import numpy as np

T_GRID = 2048
B = 16
N = 2048
NCORES = 8
BLOC = B // NCORES
NBLK_E = 16
WBLK_E = T_GRID // NBLK_E   # 128
NBLK_D = 8
TGTU = T_GRID // NBLK_D     # 256
ETH = 7.5                   # exponent cutoff; entries below e^-ETH dropped
RD = 12                     # decoder kgen rows (2 + 5*2 per half)
TP = T_GRID + 8             # padded stack width (data at col j+4-o)

_PROG_CACHE = {}


def build_program(cfg):
    import concourse.bacc as bacc
    import concourse.tile as tile
    from concourse import mybir

    f32 = mybir.dt.float32
    f16 = mybir.dt.float16
    AF = mybir.ActivationFunctionType
    OP = mybir.AluOpType

    NCH_E = cfg["NCH_E"]
    NCH_D = cfg["NCH_D"]
    J0S = cfg["J0S"]
    os_rho = cfg["os_rho"]
    b4_0 = cfg["b4_0"]
    b4_1 = cfg["b4_1"]
    SE = sum(NCH_E)
    MAXNE = max(NCH_E)
    MAXND = max(NCH_D)
    RE = 2 + 5 * MAXNE
    BW = MAXNE * WBLK_E                      # BDE col width
    KGWC = BW + 2 * NBLK_D * 128             # KGW cols
    XBC = NBLK_E * 128 + NBLK_D * 2 * TGTU   # XB cols
    assert MAXNE * WBLK_E <= 512 and MAXND <= 4

    nc = bacc.Bacc(None, target_bir_lowering=False)

    KGWh = nc.declare_dram_parameter("KGW", [17, KGWC], f16, isOutput=False)
    XBh = nc.declare_dram_parameter("XB", [BLOC, 17, XBC], f16, isOutput=False)
    PHIh = nc.declare_dram_parameter("PHI", [BLOC, 128, 2 * SE], f16, isOutput=False)
    WALLh = nc.declare_dram_parameter("WALL", [128, 332], f16, isOutput=False)
    BALLh = nc.declare_dram_parameter("BALL", [64, 8], f32, isOutput=False)
    TROWh = nc.declare_dram_parameter("TROW", [2, T_GRID], f16, isOutput=False)
    OUTh = nc.declare_dram_parameter("out", [BLOC, 2, T_GRID], f32, isOutput=True)

    with tile.TileContext(nc) as tc:
        with (
            tc.tile_pool(name="singles", bufs=1) as singles,
            tc.tile_pool(name="perb", bufs=2) as perb,
            tc.tile_pool(name="kpool", bufs=4) as kpool,
            tc.tile_pool(name="k2keep", bufs=1) as k2keep,
            tc.tile_pool(name="small", bufs=1) as small,
            tc.tile_pool(name="psE", bufs=4, space="PSUM") as psE,
            tc.tile_pool(name="psC", bufs=2, space="PSUM") as psC,
            tc.tile_pool(name="psH", bufs=2, space="PSUM") as psH,
        ):
            # ---- loads: split into queue-parallel pieces; a small first
            # piece covers the critical path (enc block 0 + PHI + TDB) ----
            KGW = singles.tile([17, KGWC], f16)
            st = [dict() for _ in range(BLOC)]
            for b in range(BLOC):
                s = st[b]
                s["XB"] = perb.tile([17, XBC], f16, tag="XB", name="XB_sb")
                s["PHI"] = perb.tile([128, 2 * SE], f16, tag="PHI", name="PHI_sb")
                s["h"] = perb.tile([2, T_GRID], f32, tag="h_sb", name="h_sb")
                s["hg0"] = perb.tile([8, TGTU], f32, tag="hg0", name="hg0")
                s["hg1"] = perb.tile([8, TGTU], f32, tag="hg1", name="hg1")
                s["rec"] = perb.tile([8, TGTU], f32, tag="rec", name="rec")
                s["h0f"] = perb.tile([8, TGTU], f16, tag="h0f", name="h0f")
                s["ratf"] = perb.tile([8, TGTU], f16, tag="ratf", name="ratf")
                s["fT"] = perb.tile([128, 2, 16], f16, tag="fT", name="fT")
                s["osl"] = perb.tile([2, T_GRID], f32, tag="osl", name="osl")
            NE = NBLK_E * 128
            nc.sync.dma_start(out=KGW[0:17, 0:BW], in_=KGWh[0:17, 0:BW])
            nc.sync.dma_start(out=st[0]["XB"][0:17, 0:512],
                              in_=XBh[0, 0:17, 0:512])
            nc.sync.dma_start(out=st[0]["PHI"], in_=PHIh[0])
            nc.sync.dma_start(out=KGW[0:12, BW:BW + 1024],
                              in_=KGWh[0:12, BW:BW + 1024])
            nc.sync.dma_start(out=st[0]["XB"][0:12, NE:NE + 2048],
                              in_=XBh[0, 0:12, NE:NE + 2048])
            nc.sync.dma_start(out=st[0]["XB"][0:17, 512:NE],
                              in_=XBh[0, 0:17, 512:NE])
            nc.sync.dma_start(out=KGW[0:12, BW + 1024:KGWC],
                              in_=KGWh[0:12, BW + 1024:KGWC])
            nc.sync.dma_start(out=st[0]["XB"][0:12, NE + 2048:XBC],
                              in_=XBh[0, 0:12, NE + 2048:XBC])
            nc.scalar.dma_start(out=st[1]["XB"][0:17, 0:NE],
                                in_=XBh[1, 0:17, 0:NE])
            nc.scalar.dma_start(out=st[1]["PHI"], in_=PHIh[1])
            WALL = singles.tile([128, 332], f16)
            nc.scalar.dma_start(out=WALL, in_=WALLh[:, :])
            nc.scalar.dma_start(out=st[1]["XB"][0:12, NE:NE + 2048],
                                in_=XBh[1, 0:12, NE:NE + 2048])
            nc.scalar.dma_start(out=st[1]["XB"][0:12, NE + 2048:XBC],
                                in_=XBh[1, 0:12, NE + 2048:XBC])
            BALL = singles.tile([64, 8], f32)
            nc.scalar.dma_start(out=BALL, in_=BALLh[:, :])

            def bde(rows, c0, c1):
                return KGW[0:rows, c0:c1]

            def tdb(half, rows, k):
                o = BW + 1024 * half
                return KGW[0:rows, o + 128 * k:o + 128 * (k + 1)]

            def xcb(b, rows, k):
                return st[b]["XB"][0:rows, 128 * k:128 * (k + 1)]

            def xtq(b, rows, k, tot):
                o = NBLK_E * 128 + 2 * TGTU * k
                return st[b]["XB"][0:rows, o:o + tot]

            W1n = WALL[0:100, 0:32]
            W1e = WALL[0:4, 32:64]
            W2n = WALL[0:128, 64:128]
            W2e = WALL[0:32, 128:192]
            W3n = WALL[0:128, 192:224]
            W3e = WALL[0:64, 224:320]
            W4n = WALL[0:128, 320:324]
            W4e = WALL[0:32, 324:328]
            ID4 = WALL[0:4, 328:332]
            B2a = BALL[0:64, 0:1]
            B3a = BALL[0:32, 1:2]
            C1a = BALL[0:32, 2:3]
            CRa = BALL[0:32, 3:7]

            # conv stacks (shared by both batches; taps in partition blocks;
            # block 0 rows 0-3 = data so shift copies read from base 0,
            # rows 4-5 of block 0 = the static affine t rows)
            C1S = singles.tile([100, TP], f16)
            nc.vector.memset(C1S, 0.0)
            nc.sync.dma_start(out=C1S[4:6, 2:2 + T_GRID], in_=TROWh[:, :])
            F2 = singles.tile([128, TP], f16)   # 4 taps x (16ch x 2b)
            F3 = singles.tile([128, TP], f16)   # 2 taps x (32ch x 2b)
            F4 = singles.tile([128, TP], f16)   # 4 taps x (16ch x 2b)
            for F, blk in ((F2, 32), (F3, 64), (F4, 32)):
                for o in range(128 // blk):
                    nc.vector.memset(F[blk * o:blk * o + blk, 0:4], 0.0)
                    nc.vector.memset(F[blk * o:blk * o + blk, T_GRID:TP], 0.0)
            FRAW = singles.tile([4, T_GRID], f16)  # b0mu,b0sg,b1mu,b1sg

            def enc_block(b, k):
                s = st[b]
                nch = NCH_E[k]
                base = sum(NCH_E[:k])
                rows = 2 + 5 * nch
                tot = nch * WBLK_E
                eps = psE.tile([128, 512], f32, tag="E", name="E_ps")
                nc.tensor.matmul(eps[:, 0:tot], xcb(b, rows, k),
                                 bde(rows, 0, tot), start=True, stop=True)
                kt = kpool.tile([128, BW], f16, tag="K", name="K1t")
                nc.scalar.activation(out=kt[:, 0:tot], in_=eps[:, 0:tot],
                                     func=AF.Exp)
                hps = psH.tile([2, TGTU], f32, tag="hms", name="h_ps")
                for c in range(nch):
                    nc.tensor.matmul(
                        hps[:, 0:WBLK_E],
                        s["PHI"][:, 2 * (base + c):2 * (base + c) + 2],
                        kt[:, WBLK_E * c:WBLK_E * (c + 1)],
                        start=(c == 0), stop=(c == nch - 1),
                    )
                nc.vector.tensor_copy(
                    s["h"][:, WBLK_E * k:WBLK_E * (k + 1)], hps[:, 0:WBLK_E])

            def dec_half(b, k, half):
                s = st[b]
                nch = min(2, NCH_D[k] - 2 * half)
                rows = 2 + 5 * nch
                tot = nch * TGTU
                if half == 0:
                    s[f"k2t_{k}"] = k2keep.tile(
                        [128, MAXND * TGTU], f16, tag=f"k2_{b}_{k}",
                        name=f"k2_{b}_{k}")
                eps = psE.tile([128, 512], f32, tag="E", name="E_ps")
                nc.tensor.matmul(eps[:, 0:tot], tdb(half, rows, k),
                                 xtq(b, rows, k, tot), start=True, stop=True)
                nc.scalar.activation(
                    out=s[f"k2t_{k}"][:, 2 * half * TGTU:2 * half * TGTU + tot],
                    in_=eps[:, 0:tot], func=AF.Exp)

            def epilogue(b):
                # gpsimd-issued DMAs: descriptor generation runs on the idle
                # software DGE instead of serializing on the sync engine
                s = st[b]
                nc.gpsimd.dma_start(out=s["hg0"], in_=s["h"][0:1, :])
                nc.gpsimd.dma_start(out=s["hg1"], in_=s["h"][1:2, :])
                nc.vector.reciprocal_approx_fast(s["rec"], s["hg0"])
                nc.vector.tensor_mul(s["ratf"], s["hg1"], s["rec"])
                nc.vector.tensor_copy(s["h0f"], s["hg0"])
                nc.gpsimd.dma_start(
                    out=C1S[2 * b:2 * b + 1, 4:4 + T_GRID], in_=s["h0f"])
                nc.gpsimd.dma_start(
                    out=C1S[2 * b + 1:2 * b + 2, 4:4 + T_GRID], in_=s["ratf"])

            def conv_chunk(l, n):
                c0 = 512 * n
                if l == 0:
                    ps = psC.tile([32, 512], f32, tag="c", name="c_ps")
                    nc.tensor.matmul(ps, W1n, C1S[:, 2 + c0:2 + c0 + 512],
                                     start=True, stop=False)
                    nc.tensor.matmul(ps, W1e, C1S[0:4, 6 + c0:6 + c0 + 512],
                                     start=False, stop=True)
                    if n == 0:
                        nc.vector.tensor_add(ps[:, 0:2], ps[:, 0:2], CRa[:, 0:2])
                    if n == 3:
                        nc.vector.tensor_add(ps[:, 510:512], ps[:, 510:512],
                                             CRa[:, 2:4])
                    nc.scalar.activation(out=F2[0:32, 4 + c0:4 + c0 + 512],
                                         in_=ps, func=AF.Relu, bias=C1a)
                elif l == 1:
                    ps = psC.tile([64, 512], f32, tag="c", name="c_ps")
                    nc.tensor.matmul(ps, W2n, F2[:, 2 + c0:2 + c0 + 512],
                                     start=True, stop=False)
                    nc.tensor.matmul(ps, W2e, F2[0:32, 6 + c0:6 + c0 + 512],
                                     start=False, stop=True)
                    nc.scalar.activation(out=F3[0:64, 4 + c0:4 + c0 + 512],
                                         in_=ps, func=AF.Relu, bias=B2a)
                elif l == 2:
                    ps = psC.tile([32, 512], f32, tag="c", name="c_ps")
                    nc.tensor.matmul(ps, W3n, F3[:, 2 + c0:2 + c0 + 512],
                                     start=True, stop=False)
                    for o in (2, 3, 4):
                        nc.tensor.matmul(
                            ps, W3e[:, 32 * (o - 2):32 * (o - 1)],
                            F3[0:64, 2 + c0 + o:2 + c0 + o + 512],
                            start=False, stop=(o == 4))
                    nc.scalar.activation(out=F4[0:32, 4 + c0:4 + c0 + 512],
                                         in_=ps, func=AF.Relu, bias=B3a)
                else:
                    ps = psC.tile([4, 512], f32, tag="c", name="c_ps")
                    nc.tensor.matmul(ps, W4n, F4[:, 2 + c0:2 + c0 + 512],
                                     start=True, stop=False)
                    nc.tensor.matmul(ps, W4e, F4[0:32, 6 + c0:6 + c0 + 512],
                                     start=False, stop=True)
                    nc.vector.tensor_copy(FRAW[:, c0:c0 + 512], ps)

            def stack_shift(F, blk, rows, n):
                # per-chunk tap-block shifts so the next layer can start
                # before this layer's later chunks finish
                c0 = 512 * n
                for o in range(1, 128 // blk):
                    nc.vector.tensor_copy(
                        F[blk * o:blk * o + rows,
                          4 + c0 - o:4 + c0 + 512 - o],
                        F[0:rows, 4 + c0:4 + c0 + 512])

            # softplus via exp + ln1p(u) minimax poly keeps Scalar inside
            # the Exp/Relu act table (no mid-program ACT_TABLE_LOAD)
            LN1P = (-0.055459313742069534, 0.21866548366220714,
                    -0.46644243862756585, 0.9962619482337954,
                    6.944574454161809e-05)
            MUB = singles.tile([128, 16], f32)
            nc.vector.memset(MUB, float(os_rho * b4_0))

            def fchain(b):
                s = st[b]
                ftp = psC.tile([128, 64], f16, tag="c", name="ftp")
                for j in range(16):
                    nc.tensor.transpose(
                        ftp[:, 4 * j:4 * j + 4],
                        FRAW[:, 128 * j:128 * (j + 1)],
                        ID4)
                mu = ftp[:, 2 * b::4]
                sg = ftp[:, 2 * b + 1::4]
                x = small.tile([128, 16], f32, tag="t1", name="t1")
                e = small.tile([128, 16], f32, tag="t2", name="t2")
                p = small.tile([128, 16], f32, tag="t3", name="t3")
                r = small.tile([128, 16], f32, tag="t4", name="t4")
                nc.vector.scalar_tensor_tensor(
                    s["fT"][:, 0, :], mu, float(os_rho), MUB,
                    OP.mult, OP.add)
                nc.vector.tensor_scalar_add(x, sg, float(b4_1))
                nc.vector.scalar_tensor_tensor(e, x, -1.0, x, OP.mult, OP.min)
                nc.scalar.activation(out=e, in_=e, func=AF.Exp)
                # os_rho * ln1p(e) via nested Horner, one STT per step
                cs = [float(os_rho * c) for c in LN1P]
                nc.vector.tensor_scalar_mul(p, e, cs[0])
                for ck in cs[1:-1]:
                    nc.vector.scalar_tensor_tensor(p, p, ck, e,
                                                   OP.add, OP.mult)
                nc.vector.tensor_scalar_add(p, p, cs[-1])
                nc.vector.tensor_scalar(r, x, float(os_rho), 0.0,
                                        OP.mult, OP.max)
                nc.vector.tensor_add(s["fT"][:, 1, :], p, r)

            def dec_mm(b, k):
                s = st[b]
                kt = s[f"k2t_{k}"]
                nch = NCH_D[k]
                msps = psH.tile([2, TGTU], f32, tag="hms", name="ms_ps")
                for c in range(nch):
                    nc.tensor.matmul(
                        msps,
                        s["fT"][:, :, J0S[k] + c],
                        kt[:, TGTU * c:TGTU * (c + 1)],
                        start=(c == 0), stop=(c == nch - 1),
                    )
                nc.vector.tensor_copy(
                    s["osl"][:, TGTU * k:TGTU * (k + 1)], msps)
                if k == NBLK_D - 1:
                    nc.sync.dma_start(out=OUTh[b], in_=s["osl"])

            # ---------------- emission ----------------
            dec_units = [(b, k, h) for b in range(BLOC)
                         for k in range(NBLK_D)
                         for h in range(2) if 2 * h < NCH_D[k]]
            du = [0]

            def emit_dec(nu=1):
                for _ in range(nu):
                    if du[0] < len(dec_units):
                        b, k, h = dec_units[du[0]]
                        dec_half(b, k, h)
                        du[0] += 1

            for k in range(NBLK_E):
                enc_block(0, k)
                if k >= 5 and k % 2 == 1:
                    emit_dec(1)
            epilogue(0)
            for k in range(NBLK_E):
                enc_block(1, k)
                emit_dec(1)
            epilogue(1)

            nexts = {0: (F2, 32, 32), 1: (F3, 64, 64), 2: (F4, 32, 32)}
            for n in range(4):
                stack_shift(C1S, 32, 4, n)
            for l in range(4):
                for n in range(4):
                    conv_chunk(l, n)
                    if l < 3 and n >= 1:
                        stack_shift(*nexts[l][:2], nexts[l][2], n - 1)
                    emit_dec(1)
                if l < 3:
                    stack_shift(*nexts[l][:2], nexts[l][2], 3)
            emit_dec(len(dec_units))    # drain any remainder

            fchain(0)
            fchain(1)
            for k in range(NBLK_D):
                dec_mm(0, k)
                dec_mm(1, k)

    nc.compile()
    return nc


def _hi_lo(vals):
    """Split into f16-exact hi (multiples of 1/16) + small f16 lo."""
    f16, f64 = np.float16, np.float64
    hi = (np.round(np.asarray(vals, f64) * 16.0) / 16.0).astype(f16)
    lo = (np.asarray(vals, f64) - hi.astype(f64)).astype(f16)
    return hi, lo


def make_inmaps(inputs):
    f32 = np.float32
    f16 = np.float16
    f64 = np.float64
    xc = np.asarray(inputs["xc"])[..., 0].astype(f32)
    yc = np.asarray(inputs["yc"])[..., 0].astype(f32)
    xt = np.asarray(inputs["xt"])[..., 0].astype(f32)
    ls_psi = f64(np.float32(inputs["ls_psi"]))
    os_psi = f64(np.float32(inputs["os_psi"]))
    ls_rho = f64(np.float32(inputs["ls_rho"]))
    os_rho = f64(np.float32(inputs["os_rho"]))
    w = [np.asarray(inputs[f"w{i}"]).astype(f32) for i in (1, 2, 3, 4)]
    bs = [np.asarray(inputs[f"b{i}"]).astype(f32) for i in (1, 2, 3, 4)]

    lower = np.minimum(xc.min(), xt.min())
    upper = np.maximum(xc.max(), xt.max())
    t64 = np.linspace(f64(lower), f64(upper), T_GRID)
    delta = (t64[-1] - t64[0]) / (T_GRID - 1)

    a_psi = -0.5 / (ls_psi * ls_psi)
    a_rho = -0.5 / (ls_rho * ls_rho)
    m_psi = np.sqrt(ETH / -a_psi)
    m_rho = np.sqrt(ETH / -a_rho)

    perm_c = np.argsort(xc, axis=1, kind="stable")
    xcs = np.take_along_axis(xc, perm_c, 1).astype(f64)
    ycs = np.take_along_axis(yc, perm_c, 1).astype(f64)
    perm_t = np.argsort(xt, axis=1, kind="stable")
    xts = np.take_along_axis(xt, perm_t, 1).astype(f64)

    # encoder windows (16 blocks of 128 grid points)
    eidx = np.zeros((B, NBLK_E, 2), np.int64)
    for k in range(NBLK_E):
        lo = t64[WBLK_E * k] - m_psi
        hi = t64[WBLK_E * (k + 1) - 1] + m_psi
        for b in range(B):
            eidx[b, k, 0] = np.searchsorted(xcs[b], lo)
            eidx[b, k, 1] = np.searchsorted(xcs[b], hi)
    ecnt = eidx[:, :, 1] - eidx[:, :, 0]
    NCH_E = [max(1, int(np.ceil(ecnt[:, k].max() / 128)))
             for k in range(NBLK_E)]
    assert max(NCH_E) <= 4, NCH_E

    # decoder grid-chunk windows per xt quantile-block
    J0S, J1S = [], []
    for k in range(NBLK_D):
        xmin = min(xts[b, TGTU * k] for b in range(B))
        xmax = max(xts[b, TGTU * (k + 1) - 1] for b in range(B))
        g0 = max(0, int(np.searchsorted(t64, xmin - m_rho)) - 1)
        g1 = min(T_GRID - 1, int(np.searchsorted(t64, xmax + m_rho)))
        J0S.append(g0 // 128)
        J1S.append(g1 // 128 + 1)
    NCH_D = [J1S[k] - J0S[k] for k in range(NBLK_D)]
    assert max(NCH_D) <= 4, NCH_D
    SE = sum(NCH_E)
    MAXNE = max(NCH_E)
    RE = 2 + 5 * MAXNE
    BW = MAXNE * WBLK_E
    KGWC = BW + 2 * NBLK_D * 128
    XBC = NBLK_E * 128 + NBLK_D * 2 * TGTU

    tpr = (np.arange(WBLK_E) - (WBLK_E - 1) / 2.0) * delta
    te2_hi, te2_lo = _hi_lo(a_psi * tpr * tpr)
    th_hi, th_lo = _hi_lo(tpr)

    # KGW: [BDE | TDB0 | TDB1]
    KGW = np.zeros((17, KGWC), f16)
    for c in range(MAXNE):
        sl = slice(WBLK_E * c, WBLK_E * (c + 1))
        KGW[0, sl] = te2_hi
        KGW[1, sl] = te2_lo
        KGW[2 + 5 * c, sl] = 1
        KGW[3 + 5 * c, sl] = 1
        KGW[4 + 5 * c, sl] = th_hi
        KGW[5 + 5 * c, sl] = th_lo
        KGW[6 + 5 * c, sl] = th_hi
    for k in range(NBLK_D):
        gv = t64[128 * J0S[k]:128 * J1S[k]]
        cb = (gv[0] + gv[-1]) / 2.0
        tv = gv - cb
        for half in range(2):
            o = BW + 1024 * half
            ksl = slice(o + 128 * k, o + 128 * (k + 1))
            KGW[0:2, ksl] = 1
        for c in range(NCH_D[k]):
            half, cc = divmod(c, 2)
            o = BW + 1024 * half
            ksl = slice(o + 128 * k, o + 128 * (k + 1))
            tvc = tv[128 * c:128 * (c + 1)]
            gb_hi, gb_lo = _hi_lo(a_rho * tvc * tvc)
            v_hi, v_lo = _hi_lo(-2.0 * a_rho * tvc)
            KGW[2 + 5 * cc, ksl] = gb_hi
            KGW[3 + 5 * cc, ksl] = gb_lo
            KGW[4 + 5 * cc, ksl] = v_hi
            KGW[5 + 5 * cc, ksl] = v_hi
            KGW[6 + 5 * cc, ksl] = v_lo

    # conv1 t channel: affine in t -> 2 static rows + bias + edge fix
    t_hi, t_lo = _hi_lo(t64)
    TROW = np.stack([t_hi, t_lo], 0)
    A1 = w[0][:, 0, :].astype(f64).sum(1)
    C1 = bs[0].astype(f64) + delta * (w[0][:, 0, :].astype(f64)
                                      * (np.arange(5) - 2)).sum(1)
    L, U = t64[0], t64[-1]
    CR = np.zeros((32, 4), f64)
    w10 = w[0][:, 0, :].astype(f64)
    for half in range(2):
        r = slice(16 * half, 16 * half + 16)
        CR[r, 0] = -w10[:, 0] * (L - 2 * delta) - w10[:, 1] * (L - delta)
        CR[r, 1] = -w10[:, 0] * (L - delta)
        CR[r, 2] = -w10[:, 4] * (U + delta)
        CR[r, 3] = -w10[:, 3] * (U + delta) - w10[:, 4] * (U + 2 * delta)

    # block-diagonal batched conv weights, packed into WALL [128, 332]
    W1n = np.zeros((128, 32), f16)
    W1n[4, :] = np.tile(A1.astype(f16), 2)
    W1n[5, :] = np.tile(A1.astype(f16), 2)
    for o in range(4):
        for half in range(2):
            W1n[32 * o + 2 * half, 16 * half:16 * half + 16] = \
                w[0][:, 1, o].astype(f16)
            W1n[32 * o + 1 + 2 * half, 16 * half:16 * half + 16] = \
                w[0][:, 2, o].astype(f16)
    W1e = np.zeros((4, 32), f16)
    for half in range(2):
        W1e[2 * half, 16 * half:16 * half + 16] = w[0][:, 1, 4].astype(f16)
        W1e[1 + 2 * half, 16 * half:16 * half + 16] = w[0][:, 2, 4].astype(f16)
    W2n = np.zeros((128, 64), f16)
    for o in range(4):
        for half in range(2):
            W2n[32 * o + 16 * half:32 * o + 16 * half + 16,
                32 * half:32 * half + 32] = w[1][:, :, o].T.astype(f16)
    W2e = np.zeros((32, 64), f16)
    for half in range(2):
        W2e[16 * half:16 * half + 16, 32 * half:32 * half + 32] = \
            w[1][:, :, 4].T.astype(f16)
    W3n = np.zeros((128, 32), f16)
    for o in range(2):
        for half in range(2):
            W3n[64 * o + 32 * half:64 * o + 32 * half + 32,
                16 * half:16 * half + 16] = w[2][:, :, o].T.astype(f16)
    W3e = np.zeros((64, 96), f16)
    for o in (2, 3, 4):
        for half in range(2):
            W3e[32 * half:32 * half + 32,
                32 * (o - 2) + 16 * half:32 * (o - 2) + 16 * half + 16] = \
                w[2][:, :, o].T.astype(f16)
    W4n = np.zeros((128, 4), f16)
    for o in range(4):
        for half in range(2):
            W4n[32 * o + 16 * half:32 * o + 16 * half + 16,
                2 * half:2 * half + 2] = w[3][:, :, o].T.astype(f16)
    W4e = np.zeros((32, 4), f16)
    for half in range(2):
        W4e[16 * half:16 * half + 16, 2 * half:2 * half + 2] = \
            w[3][:, :, 4].T.astype(f16)
    WALL = np.zeros((128, 332), f16)
    WALL[0:128, 0:32] = W1n
    WALL[0:4, 32:64] = W1e
    WALL[0:128, 64:128] = W2n
    WALL[0:32, 128:192] = W2e
    WALL[0:128, 192:224] = W3n
    WALL[0:64, 224:320] = W3e
    WALL[0:128, 320:324] = W4n
    WALL[0:32, 324:328] = W4e
    WALL[0:4, 328:332] = np.eye(4, dtype=f16)

    BALL = np.zeros((64, 8), f32)
    BALL[0:64, 0] = np.concatenate([bs[1], bs[1]])
    BALL[0:32, 1] = np.concatenate([bs[2], bs[2]])
    BALL[0:32, 2] = np.concatenate([C1, C1]).astype(f32)
    BALL[0:32, 3:7] = CR.astype(f32)

    shared = {"KGW": KGW, "WALL": WALL, "BALL": BALL, "TROW": TROW}

    in_maps = []
    for core in range(NCORES):
        m = dict(shared)
        XB = np.zeros((BLOC, 17, XBC), f16)
        PHI = np.zeros((BLOC, 128, 2 * SE), f16)
        for bb in range(BLOC):
            b = core * BLOC + bb
            XB[bb, 0:2, 0:NBLK_E * 128] = 1
            base = 0
            for k in range(NBLK_E):
                ck = (t64[WBLK_E * k] + t64[WBLK_E * (k + 1) - 1]) / 2.0
                i0, i1 = eidx[b, k]
                nv = int(i1 - i0)
                ns = 128 * NCH_E[k]
                xv = np.zeros(ns, f64)
                xv[:nv] = xcs[b, i0:i1] - ck
                bias = np.full(ns, -60.0, f64)
                bias[:nv] = a_psi * xv[:nv] * xv[:nv]
                uv = np.zeros(ns, f64)
                uv[:nv] = -2.0 * a_psi * xv[:nv]
                ph = np.zeros((ns, 2), f64)
                ph[:nv, 0] = os_psi
                ph[:nv, 1] = os_psi * ycs[b, i0:i1]
                ksl = slice(128 * k, 128 * (k + 1))
                for c in range(NCH_E[k]):
                    sl = slice(128 * c, 128 * (c + 1))
                    b_hi, b_lo = _hi_lo(bias[sl])
                    u_hi, u_lo = _hi_lo(uv[sl])
                    XB[bb, 2 + 5 * c, ksl] = b_hi
                    XB[bb, 3 + 5 * c, ksl] = b_lo
                    XB[bb, 4 + 5 * c, ksl] = u_hi
                    XB[bb, 5 + 5 * c, ksl] = u_hi
                    XB[bb, 6 + 5 * c, ksl] = u_lo
                    PHI[bb, :, 2 * (base + c)] = ph[sl, 0].astype(f16)
                    PHI[bb, :, 2 * (base + c) + 1] = ph[sl, 1].astype(f16)
                base += NCH_E[k]
            for k in range(NBLK_D):
                gv = t64[128 * J0S[k]:128 * J1S[k]]
                cb = (gv[0] + gv[-1]) / 2.0
                i0, i1 = TGTU * k, TGTU * (k + 1)
                assert xts[b, i0] - m_rho >= gv[0] - delta or J0S[k] == 0
                assert xts[b, i1 - 1] + m_rho <= gv[-1] + delta \
                    or J1S[k] == 16
                xv = xts[b, i0:i1] - cb
                xb_hi, xb_lo = _hi_lo(a_rho * xv * xv)
                x_hi, x_lo = _hi_lo(xv)
                k0 = NBLK_E * 128 + 2 * TGTU * k
                for cc in range(2):
                    csl = slice(k0 + TGTU * cc, k0 + TGTU * (cc + 1))
                    XB[bb, 0, csl] = xb_hi
                    XB[bb, 1, csl] = xb_lo
                    XB[bb, 2 + 5 * cc, csl] = 1
                    XB[bb, 3 + 5 * cc, csl] = 1
                    XB[bb, 4 + 5 * cc, csl] = x_hi
                    XB[bb, 5 + 5 * cc, csl] = x_lo
                    XB[bb, 6 + 5 * cc, csl] = x_hi
        m["XB"] = XB
        m["PHI"] = PHI
        in_maps.append(m)

    cfg = {
        "NCH_E": NCH_E, "NCH_D": NCH_D, "J0S": J0S,
        "os_rho": float(os_rho), "b4_0": float(bs[3][0]),
        "b4_1": float(bs[3][1]),
    }
    aux = {"perm_t": perm_t}
    return in_maps, cfg, aux


def kernel(**inputs):
    from concourse.bass_utils import run_bass_kernel_spmd

    in_maps, cfg, aux = make_inmaps(inputs)
    key = (tuple(cfg["NCH_E"]), tuple(cfg["NCH_D"]), tuple(cfg["J0S"]),
           cfg["os_rho"], cfg["b4_0"], cfg["b4_1"])
    if key not in _PROG_CACHE:
        _PROG_CACHE[key] = build_program(cfg)
    nc = _PROG_CACHE[key]

    res = run_bass_kernel_spmd(nc, in_maps, core_ids=list(range(NCORES)))
    outs = [np.asarray(res.results[i]["out"]) for i in range(NCORES)]
    packed = np.concatenate(outs, 0)  # [B, 2, N] in sorted-xt order
    out = np.zeros((B, N, 2), np.float32)
    for b in range(B):
        out[b, aux["perm_t"][b], 0] = packed[b, 0]
        out[b, aux["perm_t"][b], 1] = packed[b, 1]
    return out
